# revision 1
# baseline (speedup 1.0000x reference)
"""Trainium2 Bass kernel for nn_DiagGaussianActor (MoE-routing actor MLP).

Data-parallel over 8 NeuronCores: batch 8192 is split into 8 shards of
1024; gate + expert weights are replicated. Per core, the blended-expert
MLP runs with all GEMMs on the tensor engine in bf16 (fp32 PSUM
accumulation):

  - activations kept feature-major [feat, batch]; N=512 matmuls
  - the gate MLP runs in bf16 (FWL hides LDWEIGHTS under the matmuls)
  - per-layer expert blending folded into the GEMM: inputs are scaled by
    broadcast blend tiles (xs_p = x * blend[p,:], bf16 2x-mode on DVE)
    and all 8 experts accumulate into one PSUM bank; the per-sample
    blended bias is added with a K=8 matmul against blend_fm [8, batch]
  - the final layer is pair-stacked feature-major; experts are combined
    by a partition-sliced blend multiply (DVE) and a [I64;I64] selector
    matmul that folds the two 64-row halves straight into a PSUM bank
    pre-loaded with the blended bias
  - weights are repacked host-side into [128, W] tensors with multi-KB
    contiguous per-partition lines and DMA'd in consumption order
  - the two 512-column macro-batches are software-pipelined: macro 1's
    gate matmuls cover macro 0's softmax/blend latency, and macro 0's
    final layer is interleaved into macro 1's hidden-layer matmuls, so
    the PE never idles long enough to re-throttle (HAM)
"""
import sys

sys.path.insert(0, "/opt/trn_rl_repo")

import numpy as np

import concourse.bass as bass
import concourse.mybir as mybir
import concourse.tile as tile
from concourse.vector_clock import ScopedClock, VectorClock

F32 = mybir.dt.float32
F32R = mybir.dt.float32r
BF16 = mybir.dt.bfloat16
AF = mybir.ActivationFunctionType
ALU = mybir.AluOpType

B = 8192
OBS = 256
ACT2 = 64  # 2 * action_dim
HID = 512
P = 8
NCORES = 8
BL = B // NCORES          # batch per core = 1024
NB = 512                  # macro-batch (free-dim) size
NMACRO = BL // NB         # 2
KO = OBS // 128           # 2 obs k-chunks
KH = HID // 128           # 4 hidden k-chunks
NO = HID // 128           # 4 output chunks for HID layers
NWARM = 110               # HAM warm-up matmuls (N=64)


class _SplitDrainTileContext(tile.TileContext):
    """The walrus build in this container accepts very few sync-wait
    commands per instruction; the stock kernel-tail drain carries one wait
    per logical proc and fails codegen. Emit one SP nop per proc instead."""

    def _drain_and_barrier(self, tick_clock, wait_clock):
        gc = tick_clock.global_clock
        vec = list(gc)
        n = len(vec)
        for i, t in enumerate(vec):
            if t <= 0:
                continue
            sub = VectorClock([vec[j] if j == i else 0 for j in range(n)])
            nop = self.nc.sync.nop(nofuse=True)
            wait_clock.add_sem_waits(nop.ins, ScopedClock({None: sub}))
        self.nc.sync.drain()
        self.nc.all_engine_barrier()
        assert self.sems is not None
        popped = self.nc._tile_sem_poison_stack.pop()
        assert popped is self._sem_poison
        self.nc.clear_and_free_semaphores(list(self.sems.allocated().values()))
        self.nc.all_engine_barrier()


def _split_multi_waits(nc):
    """Hoist all but one sync-wait of each instruction onto NoOps on the
    same engine placed immediately before it (same queue => order kept)."""
    for f in nc.m.functions:
        for bb in f.blocks:
            new_insts = []
            for inst in bb.instructions:
                si = getattr(inst, "sync_info", None)
                ow = list(getattr(si, "on_wait", None) or [])
                if len(ow) > 1:
                    for w in ow[:-1]:
                        nop = mybir.InstNoOp(
                            name=f"I-waitsplit-{nc.next_id()}", ins=[], outs=[]
                        )
                        nop.engine = inst.engine
                        nop.sync_info = mybir.SyncInfo(on_wait=[w], on_update=[])
                        new_insts.append(nop)
                    si.on_wait = [ow[-1]]
                new_insts.append(inst)
            bb.instructions[:] = new_insts


def _build_program():
    nc = bass.Bass("TRN2", target_bir_lowering=False, debug=False)

    def din(name, shape, dtype=BF16):
        return nc.dram_tensor(name, shape, dtype, kind="ExternalInput").ap()

    # packed [128, W] weight tensors; col layout noted per tensor
    obs_w = din("obs_w", [128, KO * BL])          # chunk (m,k) at (m*KO+k)*NB
    gw0_w = din("gw0_w", [128, KO * HID])         # chunk k at cols k*HID
    gw1_w = din("gw1_w", [128, KH * HID])
    gw2_w = din("gw2_w", [128, KH * P])           # chunk k at cols k*P
    gb0_r = din("gb0_r", [128, NO], F32)          # col o = gb0[128o:128(o+1)]
    gb1_r = din("gb1_r", [128, NO], F32)
    gb2_c = din("gb2_c", [P, 1], F32)
    ew0_w = din("ew0_w", [128, KO * P * HID])     # chunk (k,p) at (k*P+p)*HID
    ew1_w = din("ew1_w", [128, KH * P * HID])
    ew2p_w = din("ew2p_w", [128, 4 * KH * 128])   # chunk (j,k) at (j*KH+k)*128
    eb0_m = din("eb0_m", [P, HID])
    eb1_m = din("eb1_m", [P, HID])
    eb2_m = din("eb2_m", [P, ACT2])
    ones8 = din("ones8", [P, 1], BF16)
    ones18 = din("ones18", [1, P], F32)
    selB = din("selB", [P, P * 128])              # selB[:, 128p:] = one-hot row p
    pairsel = din("pairsel", [P, 4 * 128])        # pair broadcast selectors
    sel64 = din("sel64", [128, ACT2])             # [I64; I64]

    out_t = nc.dram_tensor("out_t", [ACT2, BL], F32, kind="ExternalOutput").ap()

    with _SplitDrainTileContext(nc) as tc:
        with tc.tile_pool(name="wp", bufs=1) as wp, \
             tc.tile_pool(name="ap", bufs=2) as ap, \
             tc.tile_pool(name="xsp", bufs=6) as xsp, \
             tc.tile_pool(name="sp", bufs=2) as sp, \
             tc.tile_pool(name="pp", bufs=8, space="PSUM") as pp:

            # ---- load weights / consts, in consumption order ----
            def wtile(name, shape, src, dtype=BF16, nsplit=1, eng=None):
                eng = eng or nc.sync
                t = wp.tile(shape, dtype, name=name)
                w = shape[1] // nsplit
                for s in range(nsplit):
                    eng.dma_start(t[:, s * w:(s + 1) * w],
                                  src[:, s * w:(s + 1) * w])
                return t

            # gate-critical path: k-chunks alternate across the two
            # HWDGE queues (per-queue transfers are FIFO; engines shared),
            # so each gate layer can start on its first-arriving chunk.
            # GpSimd SWDGE carries the tiny consts.
            obs_sb = wp.tile([128, KO * BL], BF16, name="obs_sb")
            gw0_sb = wp.tile([128, KO * HID], BF16, name="gw0_sb")
            gw1_sb = wp.tile([128, KH * HID], BF16, name="gw1_sb")
            nc.sync.dma_start(obs_sb[:, 0:NB], obs_w[:, 0:NB])
            nc.scalar.dma_start(gw0_sb[:, 0:HID], gw0_w[:, 0:HID])
            nc.sync.dma_start(gw0_sb[:, HID:2 * HID], gw0_w[:, HID:2 * HID])
            nc.scalar.dma_start(obs_sb[:, NB:2 * NB], obs_w[:, NB:2 * NB])
            nc.sync.dma_start(gw1_sb[:, 0:HID], gw1_w[:, 0:HID])
            nc.scalar.dma_start(gw1_sb[:, HID:2 * HID], gw1_w[:, HID:2 * HID])
            nc.sync.dma_start(gw1_sb[:, 2 * HID:3 * HID],
                              gw1_w[:, 2 * HID:3 * HID])
            nc.scalar.dma_start(gw1_sb[:, 3 * HID:4 * HID],
                                gw1_w[:, 3 * HID:4 * HID])
            nc.sync.dma_start(obs_sb[:, KO * NB:KO * BL],
                              obs_w[:, KO * NB:KO * BL])
            gw2_sb = wtile("gw2_sb", [128, KH * P], gw2_w, eng=nc.gpsimd)
            gb0_sb = wtile("gb0_sb", [128, NO], gb0_r, F32, eng=nc.gpsimd)
            gb1_sb = wtile("gb1_sb", [128, NO], gb1_r, F32, eng=nc.gpsimd)
            gb2_sb = wtile("gb2_sb", [P, 1], gb2_c, F32, eng=nc.gpsimd)
            ones8_sb = wtile("ones8_sb", [P, 1], ones8, eng=nc.gpsimd)
            ones18_sb = wtile("ones18_sb", [1, P], ones18.bitcast(F32R), F32R,
                              eng=nc.gpsimd)
            eb0_sb = wtile("eb0_sb", [P, HID], eb0_m)
            selB_sb = wtile("selB_sb", [P, P * 128], selB)
            pairsel_sb = wtile("pairsel_sb", [P, 4 * 128], pairsel)
            eb1_sb = wtile("eb1_sb", [P, HID], eb1_m)
            ew0_sb = wtile("ew0_sb", [128, KO * P * HID], ew0_w, nsplit=2)
            ew1_sb = wtile("ew1_sb", [128, KH * P * HID], ew1_w, nsplit=4)
            ew2p_sb = wtile("ew2p_sb", [128, 4 * KH * 128], ew2p_w)
            sel64_sb = wtile("sel64_sb", [128, ACT2], sel64)
            eb2_sb = wtile("eb2_sb", [P, ACT2], eb2_m)

            def ew0_c(p, k, o):
                c = (k * P + p) * HID + o * 128
                return ew0_sb[:, c:c + 128]

            def ew1_c(p, k, o):
                c = (k * P + p) * HID + o * 128
                return ew1_sb[:, c:c + 128]

            def ew2p_c(j, k):
                c = (j * KH + k) * 128
                return ew2p_sb[:, c:c + 128]

            def obs_c(k, m):
                c = (m * KO + k) * NB
                return obs_sb[:, c:c + NB]

            neg15 = wp.tile([128, 1], F32, name="neg15")
            nc.vector.memset(neg15[:], -1.5)
            # HAM warm-up: keep the PE busy while the first DMAs land
            warm = wp.tile([128, 128], BF16, name="warm")
            nc.vector.memset(warm[:], 1.0)
            ps_w = pp.tile([128, 64], F32, name="ps_warm", tag="ps", bufs=4, padded_shape=[128, NB])
            for _ in range(NWARM):
                nc.tensor.matmul(ps_w[:], warm[:], warm[:, 0:64],
                                 start=True, stop=True)

            # ---------------- gate network ----------------
            def relu_evac(xt, ps, bias, o):
                # balance evacuation across ACT / DVE
                if o % 2 == 0:
                    nc.scalar.activation(xt[:], ps[:], AF.Relu, bias=bias)
                else:
                    nc.vector.tensor_scalar(xt[:], ps[:], bias, 0.0,
                                            op0=ALU.add, op1=ALU.max)

            def gate_layer(m, lid, wsb, nk, xin, gb_sb):
                # k-outer emission: first matmuls need only the k=0 weight
                # chunk, so compute can start on partially-arrived DMAs
                pss = [pp.tile([128, NB], F32, name=f"ps_g{lid}_{o}",
                               tag="acc", bufs=4) for o in range(NO)]
                for k in range(nk):
                    for o in range(NO):
                        nc.tensor.matmul(
                            pss[o][:],
                            wsb[:, k * HID + o * 128:k * HID + (o + 1) * 128],
                            xin[k][:], start=(k == 0), stop=(k == nk - 1))
                xg = []
                for o in range(NO):
                    xt = ap.tile([128, NB], BF16, name=f"xg{lid}_{m}_{o}",
                                 bufs=1)
                    nc.vector.tensor_scalar(xt[:], pss[o][:],
                                            gb_sb[:, o:o + 1], 0.0,
                                            op0=ALU.add, op1=ALU.max)
                    xg.append(xt)
                return xg

            def build_gate(m, after_g0=None, after_g1=None):
                obs_m = [obs_c(k, m) for k in range(KO)]
                xg0 = gate_layer(m, 0, gw0_sb, KO, obs_m, gb0_sb)
                if after_g0 is not None:
                    after_g0()
                xg1 = gate_layer(m, 1, gw1_sb, KH, xg0, gb1_sb)
                if after_g1 is not None:
                    after_g1()
                ps_lg = pp.tile([P, NB], F32, name=f"ps_lg_{m}", tag="ps", bufs=4)
                for k in range(KH):
                    nc.tensor.matmul(ps_lg[:], gw2_sb[:, k * P:(k + 1) * P],
                                     xg1[k][:], start=(k == 0), stop=False)
                e_fm = sp.tile([P, NB], BF16, name=f"e_fm_{m}")
                nc.scalar.activation(e_fm[:], ps_lg[:], AF.Exp, bias=gb2_sb[:])
                return ps_lg, e_fm

            # softmax tail, split so each PE piece can be emitted separately
            def blend_sum(m, e_fm):
                ps_s = pp.tile([1, NB], F32, name=f"ps_s_{m}", tag="ps", bufs=4)
                nc.tensor.matmul(ps_s[:], ones8_sb[:], e_fm[:],
                                 start=True, stop=True)
                return ps_s

            def blend_ln(m, ps_s):
                # log-softmax: lnS on ACT (~0.5us) beats the 3.3us DVE
                # single-lane iterative-divide reciprocal
                lns = sp.tile([1, NB], F32R, name=f"lns_{m}")
                with nc.allow_low_precision(reason="f32r storage for lnS"):
                    nc.scalar.activation(lns[:], ps_s[:], AF.Ln)
                return lns

            def blend_norm(m, ps_lg, lns):
                # accumulate -lnS onto the logits (neg18_sb is all -1)
                nc.tensor.matmul(ps_lg[:], ones18_sb[:], lns[:],
                                 start=False, stop=True)

            def blend_exp(m, ps_lg):
                blend_fm = sp.tile([P, NB], BF16, name=f"blend_fm_{m}")
                nc.scalar.activation(blend_fm[:], ps_lg[:], AF.Exp,
                                     bias=gb2_sb[:])
                return blend_fm

            def bcast_one(m, blend_fm, p, kind, out_list):
                # broadcast one blend row to 128 partitions via selector
                # matmul + ACT copy; emitted interleaved with layer matmuls
                # so the copy latency never paces the PE
                sel = selB_sb if kind == "b" else pairsel_sb
                ps = pp.tile([128, NB], F32, name=f"ps_bc_{kind}", tag="ps",
                             bufs=4)
                nc.tensor.matmul(ps[:], sel[:, p * 128:(p + 1) * 128],
                                 blend_fm[:], start=True, stop=True)
                bb = ap.tile([128, NB], BF16, name=f"bc{kind}_{m}_{p}",
                             bufs=1)
                nc.scalar.copy(bb[:], ps[:])
                out_list.append(bb)

            # ------------- blended expert layers -------------
            def layer_acc(lname, eb_sb, blend_fm):
                """Allocate a layer's PSUM accumulators and emit the
                blended-bias matmuls (start=True)."""
                ps_l = []
                for o in range(NO):
                    ps = pp.tile([128, NB], F32, name=f"ps_{lname}_{o}",
                                 tag="acc", bufs=4)
                    nc.tensor.matmul(ps[:], eb_sb[:, o * 128:(o + 1) * 128],
                                     blend_fm[:], start=True, stop=False)
                    ps_l.append(ps)
                return ps_l

            def build_layer(m, lname, wsel, nk, xin, blendB, ps_l,
                            interleave=None):
                """One blended expert layer: out[o] = relu(sum_p W_p^T
                (x*b_p) + blend^T b).  interleave maps (k,p) chunk index ->
                fn emitting extra work between chunks."""
                ci = 0
                for k in range(nk):
                    for p in range(P):
                        if interleave and ci in interleave:
                            interleave[ci]()
                        ci += 1
                        xs = xsp.tile([128, NB], BF16, name="xs")
                        nc.vector.tensor_tensor(
                            xs[:], xin[k][:], blendB[p][:], op=ALU.mult)
                        for o in range(NO):
                            nc.tensor.matmul(
                                ps_l[o][:], wsel(p, k, o), xs[:],
                                start=False,
                                stop=(k == nk - 1 and p == P - 1))
                xout = []
                for o in range(NO):
                    xt = ap.tile([128, NB], BF16, name=f"x_{lname[:2]}_{o}")
                    if o % 2 == 0:
                        nc.scalar.activation(xt[:], ps_l[o][:], AF.Relu)
                    else:
                        nc.vector.tensor_scalar_max(xt[:], ps_l[o][:], 0.0)
                    xout.append(xt)
                return xout

            def build_l2_head(m, x2, c0=0, cw=NB):
                """Pair-stacked final-layer GEMMs + blend multiplies over
                columns [c0, c0+cw).  Returns tmul tiles; the fold matmuls
                are emitted later."""
                cs = slice(c0, c0 + cw)
                tmuls = []
                for j in range(4):
                    ps = pp.tile([128, cw], F32, name=f"ps2_{j}", tag="acc",
                                 bufs=4, padded_shape=[128, NB])
                    for k in range(KH):
                        nc.tensor.matmul(ps[:], ew2p_c(j, k), x2[k][:, cs],
                                         start=(k == 0), stop=(k == KH - 1))
                    tm = sp.tile([128, cw], BF16, name="tmul", bufs=3,
                                 padded_shape=[128, NB])
                    nc.vector.tensor_tensor(tm[:], ps[:],
                                            bbp_all[m][j][:, cs],
                                            op=ALU.mult)
                    tmuls.append(tm)
                return tmuls

            def build_l2_fold(m, key, tmuls, blend_fm, js, c0=0, cw=NB):
                """Emit y-PSUM fold matmuls for pair indices js (0=bias)."""
                cs = slice(c0, c0 + cw)
                for j in js:
                    if j == 0:
                        y_ps[key] = pp.tile([ACT2, cw], F32,
                                            name=f"ps_y_{key}", tag="ps",
                                            bufs=4, padded_shape=[128, NB])
                        nc.tensor.matmul(y_ps[key][:], eb2_sb[:],
                                         blend_fm[:, cs],
                                         start=True, stop=False)
                    nc.tensor.matmul(y_ps[key][:], sel64_sb[:], tmuls[j][:],
                                     start=False, stop=(j == 3))

            def build_l2_tail(m, key, c0=0, cw=NB):
                yp = y_ps[key]
                mu = sp.tile([32, cw], F32, name="mu_sb", bufs=2,
                             padded_shape=[32, NB])
                nc.vector.tensor_copy(mu[:], yp[0:32, :])
                tls = sp.tile([32, cw], F32, name="tls", bufs=2,
                              padded_shape=[32, NB])
                nc.scalar.activation(tls[:], yp[32:ACT2, :], AF.Tanh)
                stdt = sp.tile([32, cw], F32, name="stdt", bufs=2,
                               padded_shape=[32, NB])
                nc.scalar.activation(stdt[:], tls[:], AF.Exp,
                                     scale=3.5, bias=neg15[0:32, :])
                gc = slice(m * NB + c0, m * NB + c0 + cw)
                nc.sync.dma_start(out_t[0:32, gc], mu[:])
                nc.sync.dma_start(out_t[32:ACT2, gc], stdt[:])

            # ---------------- schedule ----------------
            y_ps = {}
            bbp_all = {}
            hold = {}

            lg0, e0 = build_gate(0)                  # gates m0

            def _m0_sum():
                hold["s0"] = blend_sum(0, e0)

            def _m0_tail():
                hold["lns0"] = blend_ln(0, hold["s0"])
                blend_norm(0, lg0, hold["lns0"])
                hold["bf0"] = blend_exp(0, lg0)

            # gates m1 cover the m0 softmax chain
            lg1, e1 = build_gate(1, after_g0=_m0_sum, after_g1=_m0_tail)
            bf0 = hold["bf0"]
            ps_s1 = blend_sum(1, e1)
            acc_l0a = layer_acc("l0a", eb0_sb, bf0)  # claim banks pre-bcast
            lns1 = blend_ln(1, ps_s1)
            blend_norm(1, lg1, lns1)
            bf1 = blend_exp(1, lg1)
            hold["bf1"] = bf1
            # first 4 expert broadcasts for m0 up front; the rest are
            # interleaved one (matmul, copy) pair per L0 chunk
            bb0, bbp0, bb_m1, bbp1 = [], [], [], []
            bbp_all[0] = bbp0
            bbp_all[1] = bbp1
            for p in range(4):
                bcast_one(0, bf0, p, "b", bb0)
            il0 = {}
            for i in range(4):
                il0[i] = (lambda p=i + 4: bcast_one(0, bf0, p, "b", bb0))
                il0[4 + i] = (lambda j=i: bcast_one(0, bf0, j, "p", bbp0))
                il0[8 + i] = (lambda p=i: bcast_one(1, bf1, p, "b", bb_m1))
                il0[12 + i] = (lambda p=i + 4: bcast_one(1, bf1, p, "b",
                                                         bb_m1))
            il0b = {i: (lambda j=i: bcast_one(1, bf1, j, "p", bbp1))
                    for i in range(4)}

            obs0 = [obs_c(k, 0) for k in range(KO)]
            obs1 = [obs_c(k, 1) for k in range(KO)]
            x1_0 = build_layer(0, "l0a", ew0_c, KO, obs0, bb0, acc_l0a,
                               interleave=il0)
            acc = layer_acc("l0b", eb0_sb, hold["bf1"])
            x1_1 = build_layer(1, "l0b", ew0_c, KO, obs1, bb_m1, acc,
                               interleave=il0b)
            acc = layer_acc("l1a", eb1_sb, bf0)
            x2_0 = build_layer(0, "l1a", ew1_c, KH, x1_0, bb0, acc)

            tm0 = build_l2_head(0, x2_0)             # L2 m0 GEMMs
            build_l2_fold(0, "m0", tm0, bf0, [0, 1, 2])

            il1 = {2: (lambda: build_l2_fold(0, "m0", tm0, bf0, [3])),
                   4: (lambda: build_l2_tail(0, "m0"))}
            acc = layer_acc("l1b", eb1_sb, hold["bf1"])
            x2_1 = build_layer(1, "l1b", ew1_c, KH, x1_1, bb_m1, acc,
                               interleave=il1)

            # m1 final layer in two column chunks so the tmul/fold/tail
            # chain of chunk A hides under chunk B's GEMMs
            HW = NB // 2
            tmA = build_l2_head(1, x2_1, 0, HW)
            tmB = build_l2_head(1, x2_1, HW, HW)
            build_l2_fold(1, "m1a", tmA, hold["bf1"], [0, 1, 2, 3], 0, HW)
            build_l2_tail(1, "m1a", 0, HW)
            build_l2_fold(1, "m1b", tmB, hold["bf1"], [0, 1, 2, 3], HW, HW)
            build_l2_tail(1, "m1b", HW, HW)

    _split_multi_waits(nc)
    return nc


_NC_CACHE = None


def _get_program():
    global _NC_CACHE
    if _NC_CACHE is None:
        _NC_CACHE = _build_program()
    return _NC_CACHE


def _prep_core_inputs(inputs):
    import ml_dtypes
    f32 = np.float32
    bf16 = ml_dtypes.bfloat16
    obs = np.ascontiguousarray(inputs["obs"], dtype=f32)
    gw0 = np.asarray(inputs["gw0"], f32)
    gb0 = np.asarray(inputs["gb0"], f32)
    gw1 = np.asarray(inputs["gw1"], f32)
    gb1 = np.asarray(inputs["gb1"], f32)
    gw2 = np.asarray(inputs["gw2"], f32)
    gb2 = np.asarray(inputs["gb2"], f32)
    ew0 = np.asarray(inputs["ew0"], f32)
    eb0 = np.asarray(inputs["eb0"], f32)
    ew1 = np.asarray(inputs["ew1"], f32)
    eb1 = np.asarray(inputs["eb1"], f32)
    ew2 = np.asarray(inputs["ew2"], f32)
    eb2 = np.asarray(inputs["eb2"], f32)

    def chunk_cols(a, nk):
        # [nk*128, W] -> [128, nk*W] with chunk k at cols [k*W:(k+1)*W]
        return np.concatenate([a[k * 128:(k + 1) * 128, :]
                               for k in range(nk)], axis=1)

    obs_tf = np.ascontiguousarray(obs.T)                      # [OBS, B] f32
    obs_bt = obs_tf.astype(bf16)
    gw0_w = np.ascontiguousarray(chunk_cols(gw0.T, KO).astype(bf16))
    gw1_w = np.ascontiguousarray(chunk_cols(gw1.T, KH).astype(bf16))
    gw2_w = np.ascontiguousarray(chunk_cols(gw2.T, KH).astype(bf16))
    gb0_r = np.ascontiguousarray(gb0.reshape(NO, 128).T)      # [128, NO]
    gb1_r = np.ascontiguousarray(gb1.reshape(NO, 128).T)
    gb2_c = gb2.reshape(P, 1).astype(f32)
    # ew packed [128, nk*P*HID], chunk (k,p) at col (k*P+p)*HID
    ew0_t = ew0.transpose(0, 2, 1)                            # [P, OBS, HID]
    ew1_t = ew1.transpose(0, 2, 1)
    ew0_w = np.concatenate(
        [ew0_t[p, k * 128:(k + 1) * 128, :]
         for k in range(KO) for p in range(P)], axis=1).astype(bf16)
    ew1_w = np.concatenate(
        [ew1_t[p, k * 128:(k + 1) * 128, :]
         for k in range(KH) for p in range(P)], axis=1).astype(bf16)
    ew2_t = ew2.transpose(0, 2, 1)                            # [P, HID, 64]
    ew2p = [np.concatenate([ew2_t[2 * j], ew2_t[2 * j + 1]], axis=1)
            for j in range(4)]                                # [HID, 128] x4
    ew2p_w = np.concatenate(
        [ew2p[j][k * 128:(k + 1) * 128, :]
         for j in range(4) for k in range(KH)], axis=1).astype(bf16)
    ones8 = np.ones((P, 1), bf16)
    ones18 = -np.ones((1, P), f32)          # negated: accumulates -lnS
    selB = np.zeros((P, P * 128), bf16)
    for p in range(P):
        selB[p, p * 128:(p + 1) * 128] = 1.0
    pairsel = np.zeros((P, 4 * 128), bf16)
    for j in range(4):
        pairsel[2 * j, j * 128:j * 128 + ACT2] = 1.0
        pairsel[2 * j + 1, j * 128 + ACT2:(j + 1) * 128] = 1.0
    sel64 = np.zeros((128, ACT2), bf16)
    for r in range(128):
        sel64[r, r % ACT2] = 1.0

    shared = {
        "gw0_w": gw0_w, "gw1_w": gw1_w, "gw2_w": gw2_w,
        "gb0_r": gb0_r, "gb1_r": gb1_r, "gb2_c": gb2_c,
        "ew0_w": np.ascontiguousarray(ew0_w),
        "ew1_w": np.ascontiguousarray(ew1_w),
        "ew2p_w": np.ascontiguousarray(ew2p_w),
        "eb0_m": eb0.astype(bf16), "eb1_m": eb1.astype(bf16),
        "eb2_m": eb2.astype(bf16),
        "ones8": ones8, "ones18": ones18, "selB": selB, "pairsel": pairsel,
        "sel64": sel64,
    }
    in_maps = []
    for c in range(NCORES):
        im = dict(shared)
        oc = obs_bt[:, c * BL:(c + 1) * BL]                   # [OBS, BL]
        im["obs_w"] = np.ascontiguousarray(
            np.concatenate([oc[k * 128:(k + 1) * 128, m * NB:(m + 1) * NB]
                            for m in range(NMACRO) for k in range(KO)],
                           axis=1))
        in_maps.append(im)
    return in_maps


def kernel(**inputs) -> np.ndarray:
    import time

    from concourse.bass_utils import run_bass_kernel_spmd

    nc = _get_program()
    in_maps = _prep_core_inputs(inputs)
    res = None
    last_err = None
    # a freshly-compiled NEFF occasionally hits a transient
    # NRT_EXEC_UNIT_UNRECOVERABLE on its first execution; a retry succeeds
    for attempt in range(3):
        try:
            res = run_bass_kernel_spmd(nc, in_maps, core_ids=list(range(NCORES)))
            break
        except Exception as e:  # noqa: BLE001
            last_err = e
            time.sleep(2.0)
    if res is None:
        raise last_err
    out = np.concatenate(
        [res.results[c]["out_t"].T for c in range(NCORES)], axis=0)
    return np.ascontiguousarray(out, dtype=np.float32)



# revision 5
# speedup vs baseline: 1.2547x; 1.2547x over previous
"""Trainium2 Bass kernel for nn_DiagGaussianActor (MoE-routing actor MLP).

Data-parallel over 8 NeuronCores: batch 8192 is split into 8 shards of
1024; gate + expert weights are replicated. Per core, the blended-expert
MLP runs with all GEMMs on the tensor engine (fp32 PSUM accumulation):

  - activations kept feature-major [feat, batch]; N=512 matmuls
  - the gate MLP runs in bf16 (FWL hides LDWEIGHTS under the matmuls)
  - per-layer expert blending folded into the GEMM: inputs are scaled by
    broadcast blend tiles (xs_p = x * blend[p,:]) and all 8 experts
    accumulate into one PSUM bank; the per-sample blended bias is added
    with a K=8 matmul against blend_fm [8, batch]
  - the first half (k-chunks 0,1) of the middle expert layer runs in
    fp8e4m3 DoubleRow mode (2 k-planes per matmul, ~1.7x PE throughput);
    weights are pre-scaled x512 host-side, activations x32 on the L0
    evacuation, and the PSUM descaled 2^-14 on the L1 evacuation
  - the final layer is pair-stacked feature-major; experts are combined
    by partition-sliced blend multiplies (DVE, reusing the bb broadcast
    tiles) and a [I64;I64] selector matmul that folds the two 64-row
    halves straight into a PSUM bank pre-loaded with the blended bias
  - weights are repacked host-side into [128, W] tensors with multi-KB
    contiguous per-partition lines and DMA'd in consumption order over
    several queues
  - the two 512-column macro-batches are software-pipelined: macro 1's
    gate matmuls cover macro 0's softmax/blend latency, the blend
    broadcasts for m0 are emitted ahead of m1's softmax tail so L0a can
    start the moment gate m1 ends (no HAM re-throttle), and macro 0's
    final layer is interleaved into macro 1's hidden-layer matmuls
"""
import sys

sys.path.insert(0, "/opt/trn_rl_repo")

import numpy as np

import concourse.bass as bass
import concourse.mybir as mybir
import concourse.tile as tile
from concourse.vector_clock import ScopedClock, VectorClock

F32 = mybir.dt.float32
F32R = mybir.dt.float32r
BF16 = mybir.dt.bfloat16
F8E4 = mybir.dt.float8e4
AF = mybir.ActivationFunctionType
ALU = mybir.AluOpType
DR = mybir.MatmulPerfMode.DoubleRow

B = 8192
OBS = 256
ACT2 = 64  # 2 * action_dim
HID = 512
P = 8
NCORES = 8
BL = B // NCORES          # batch per core = 1024
NB = 512                  # macro-batch (free-dim) size
NMACRO = BL // NB         # 2
KO = OBS // 128           # 2 obs k-chunks
KH = HID // 128           # 4 hidden k-chunks
NO = HID // 128           # 4 output chunks for HID layers
NWARM = 92                # HAM warm-up matmuls (N=64)

USE_FP8_L1 = True         # k-chunks 0,1 of L1 in fp8 DoubleRow
XSC = 32.0                # activation scale applied on the L0 evac
WSC = 512.0               # L1 weight scale (both halves)
DESC = 1.0 / (XSC * WSC)  # L1 evac descale


class _SplitDrainTileContext(tile.TileContext):
    """The walrus build in this container accepts very few sync-wait
    commands per instruction; the stock kernel-tail drain carries one wait
    per logical proc and fails codegen. Emit one SP nop per proc instead."""

    def _drain_and_barrier(self, tick_clock, wait_clock):
        gc = tick_clock.global_clock
        vec = list(gc)
        n = len(vec)
        for i, t in enumerate(vec):
            if t <= 0:
                continue
            sub = VectorClock([vec[j] if j == i else 0 for j in range(n)])
            nop = self.nc.sync.nop(nofuse=True)
            wait_clock.add_sem_waits(nop.ins, ScopedClock({None: sub}))
        self.nc.sync.drain()
        self.nc.all_engine_barrier()
        assert self.sems is not None
        popped = self.nc._tile_sem_poison_stack.pop()
        assert popped is self._sem_poison
        self.nc.clear_and_free_semaphores(list(self.sems.allocated().values()))
        self.nc.all_engine_barrier()


def _split_multi_waits(nc):
    """Hoist all but one sync-wait of each instruction onto NoOps on the
    same engine placed immediately before it (same queue => order kept)."""
    for f in nc.m.functions:
        for bb in f.blocks:
            new_insts = []
            for inst in bb.instructions:
                si = getattr(inst, "sync_info", None)
                ow = list(getattr(si, "on_wait", None) or [])
                if len(ow) > 1:
                    for w in ow[:-1]:
                        nop = mybir.InstNoOp(
                            name=f"I-waitsplit-{nc.next_id()}", ins=[], outs=[]
                        )
                        nop.engine = inst.engine
                        nop.sync_info = mybir.SyncInfo(on_wait=[w], on_update=[])
                        new_insts.append(nop)
                    si.on_wait = [ow[-1]]
                new_insts.append(inst)
            bb.instructions[:] = new_insts


def _build_program():
    nc = bass.Bass("TRN2", target_bir_lowering=False, debug=False)

    def din(name, shape, dtype=BF16):
        return nc.dram_tensor(name, shape, dtype, kind="ExternalInput").ap()

    # packed [128, W] weight tensors; col layout noted per tensor
    obs_w = din("obs_w", [128, KO * BL])          # chunk (m,k) at (m*KO+k)*NB
    gw0_w = din("gw0_w", [128, KO * HID])         # chunk k at cols k*HID
    gw1_w = din("gw1_w", [128, KH * HID])
    gw2_w = din("gw2_w", [128, KH * P])           # chunk k at cols k*P
    gb0_r = din("gb0_r", [128, NO], F32)          # col o = gb0[128o:128(o+1)]
    gb1_r = din("gb1_r", [128, NO], F32)
    gb2_c = din("gb2_c", [P, 1], F32)
    ew0_w = din("ew0_w", [128, KO * P * HID])     # chunk (k,p) at (k*P+p)*HID
    if USE_FP8_L1:
        # pair (k0,k1) per (p,o): 256 cols at (p*NO+o)*256, scaled x512
        ew1f8_w = din("ew1f8_w", [128, P * NO * 256], F8E4)
        # chunks k2,k3: chunk (k,p) at ((k-2)*P+p)*HID, scaled x512
        ew1b_w = din("ew1b_w", [128, 2 * P * HID])
    else:
        ew1_w = din("ew1_w", [128, KH * P * HID])
    ew2p_w = din("ew2p_w", [128, 4 * KH * 128])   # chunk (j,k) at (j*KH+k)*128
    eb0_m = din("eb0_m", [P, HID])
    eb1_m = din("eb1_m", [P, HID])
    eb2_m = din("eb2_m", [P, ACT2])
    ones8 = din("ones8", [P, 1], BF16)
    ones18 = din("ones18", [1, P], F32)
    selB = din("selB", [P, P * 128])              # selB[:, 128p:] = one-hot row p
    sel64 = din("sel64", [128, ACT2])             # [I64; I64]

    out_t = nc.dram_tensor("out_t", [ACT2, BL], F32, kind="ExternalOutput").ap()

    with _SplitDrainTileContext(nc) as tc:
        with tc.tile_pool(name="wp", bufs=1) as wp, \
             tc.tile_pool(name="ap", bufs=2) as ap, \
             tc.tile_pool(name="xsp", bufs=6) as xsp, \
             tc.tile_pool(name="sp", bufs=2) as sp, \
             tc.tile_pool(name="pp", bufs=8, space="PSUM") as pp:

            # ---- load weights / consts, in consumption order ----
            def wtile(name, shape, src, dtype=BF16, nsplit=1, engs=None):
                engs = engs or [nc.sync]
                t = wp.tile(shape, dtype, name=name)
                w = shape[1] // nsplit
                for s in range(nsplit):
                    engs[s % len(engs)].dma_start(
                        t[:, s * w:(s + 1) * w], src[:, s * w:(s + 1) * w])
                return t

            # gate-critical path: the first ~1 MB is spread over four DGE
            # queues so the gate can start as soon as possible; the bulk
            # expert weights alternate the sync/scalar queues.  GpSimd
            # SWDGE carries the tiny consts.
            obs_sb = wp.tile([128, KO * BL], BF16, name="obs_sb")
            gw0_sb = wp.tile([128, KO * HID], BF16, name="gw0_sb")
            gw1_sb = wp.tile([128, KH * HID], BF16, name="gw1_sb")
            nc.sync.dma_start(gw0_sb[:, 0:HID], gw0_w[:, 0:HID])
            nc.scalar.dma_start(obs_sb[:, 0:NB], obs_w[:, 0:NB])
            nc.gpsimd.dma_start(gw0_sb[:, HID:2 * HID],
                                gw0_w[:, HID:2 * HID])
            nc.scalar.dma_start(obs_sb[:, NB:2 * NB], obs_w[:, NB:2 * NB])
            nc.sync.dma_start(gw1_sb[:, 0:HID], gw1_w[:, 0:HID])
            nc.gpsimd.dma_start(gw1_sb[:, HID:2 * HID],
                                gw1_w[:, HID:2 * HID])
            nc.sync.dma_start(gw1_sb[:, 2 * HID:3 * HID],
                              gw1_w[:, 2 * HID:3 * HID])
            nc.scalar.dma_start(gw1_sb[:, 3 * HID:4 * HID],
                                gw1_w[:, 3 * HID:4 * HID])
            nc.sync.dma_start(obs_sb[:, KO * NB:KO * BL],
                              obs_w[:, KO * NB:KO * BL])
            gw2_sb = wtile("gw2_sb", [128, KH * P], gw2_w, engs=[nc.gpsimd])
            gb0_sb = wtile("gb0_sb", [128, NO], gb0_r, F32, engs=[nc.gpsimd])
            gb1_sb = wtile("gb1_sb", [128, NO], gb1_r, F32, engs=[nc.gpsimd])
            gb2_sb = wtile("gb2_sb", [P, 1], gb2_c, F32, engs=[nc.gpsimd])
            ones8_sb = wtile("ones8_sb", [P, 1], ones8, engs=[nc.gpsimd])
            ones18_sb = wtile("ones18_sb", [1, P], ones18.bitcast(F32R), F32R,
                              engs=[nc.gpsimd])
            eb0_sb = wtile("eb0_sb", [P, HID], eb0_m, engs=[nc.scalar])
            selB_sb = wtile("selB_sb", [P, P * 128], selB, engs=[nc.scalar])
            eb1_sb = wtile("eb1_sb", [P, HID], eb1_m, engs=[nc.scalar])
            ew0_sb = wtile("ew0_sb", [128, KO * P * HID], ew0_w, nsplit=2,
                           engs=[nc.sync, nc.scalar])
            if USE_FP8_L1:
                ew1f8_sb = wtile("ew1f8_sb", [128, P * NO * 256], ew1f8_w,
                                 F8E4, nsplit=2, engs=[nc.sync, nc.scalar])
                ew1b_sb = wtile("ew1b_sb", [128, 2 * P * HID], ew1b_w,
                                nsplit=2, engs=[nc.sync, nc.scalar])
            else:
                ew1_sb = wtile("ew1_sb", [128, KH * P * HID], ew1_w, nsplit=4,
                               engs=[nc.sync, nc.scalar])
            ew2p_sb = wtile("ew2p_sb", [128, 4 * KH * 128], ew2p_w)
            sel64_sb = wtile("sel64_sb", [128, ACT2], sel64,
                             engs=[nc.scalar])
            eb2_sb = wtile("eb2_sb", [P, ACT2], eb2_m, engs=[nc.scalar])

            def ew0_c(p, k, o):
                c = (k * P + p) * HID + o * 128
                return ew0_sb[:, c:c + 128]

            def ew1b_c(p, k, o):
                # k in {2, 3}
                c = ((k - 2) * P + p) * HID + o * 128
                return ew1b_sb[:, c:c + 128]

            def ew1f8_c(p, o):
                c = (p * NO + o) * 256
                return ew1f8_sb[:, c:c + 256].rearrange(
                    "p (two f) -> p two f", two=2)

            def ew2p_c(j, k):
                c = (j * KH + k) * 128
                return ew2p_sb[:, c:c + 128]

            def obs_c(k, m):
                c = (m * KO + k) * NB
                return obs_sb[:, c:c + NB]

            neg15 = wp.tile([128, 1], F32, name="neg15")
            nc.vector.memset(neg15[:], -1.5)
            # HAM warm-up: keep the PE busy while the first DMAs land
            warm = wp.tile([128, 128], BF16, name="warm")
            nc.vector.memset(warm[:], 1.0)
            ps_w = pp.tile([128, 64], F32, name="ps_warm", tag="ps", bufs=4,
                           padded_shape=[128, NB])
            for _ in range(NWARM):
                nc.tensor.matmul(ps_w[:], warm[:], warm[:, 0:64],
                                 start=True, stop=True)

            # ---------------- gate network ----------------
            def gate_layer(m, lid, wsb, nk, xin, gb_sb):
                # k-outer emission: first matmuls need only the k=0 weight
                # chunk, so compute can start on partially-arrived DMAs
                pss = [pp.tile([128, NB], F32, name=f"ps_g{lid}_{o}",
                               tag="acc", bufs=4) for o in range(NO)]
                for k in range(nk):
                    for o in range(NO):
                        nc.tensor.matmul(
                            pss[o][:],
                            wsb[:, k * HID + o * 128:k * HID + (o + 1) * 128],
                            xin[k][:], start=(k == 0), stop=(k == nk - 1))
                xg = []
                for o in range(NO):
                    xt = ap.tile([128, NB], BF16, name=f"xg{lid}_{m}_{o}",
                                 bufs=1)
                    if o % 2 == 0:
                        nc.scalar.activation(xt[:], pss[o][:], AF.Relu,
                                             bias=gb_sb[:, o:o + 1])
                    else:
                        nc.vector.tensor_scalar(xt[:], pss[o][:],
                                                gb_sb[:, o:o + 1], 0.0,
                                                op0=ALU.add, op1=ALU.max)
                    xg.append(xt)
                return xg

            def build_gate(m, after_g0=None, after_g1=None):
                obs_m = [obs_c(k, m) for k in range(KO)]
                xg0 = gate_layer(m, 0, gw0_sb, KO, obs_m, gb0_sb)
                if after_g0 is not None:
                    after_g0()
                xg1 = gate_layer(m, 1, gw1_sb, KH, xg0, gb1_sb)
                if after_g1 is not None:
                    after_g1()
                ps_lg = pp.tile([P, NB], F32, name=f"ps_lg_{m}", tag="ps",
                                bufs=4)
                for k in range(KH):
                    nc.tensor.matmul(ps_lg[:], gw2_sb[:, k * P:(k + 1) * P],
                                     xg1[k][:], start=(k == 0), stop=False)
                e_fm = sp.tile([P, NB], BF16, name=f"e_fm_{m}")
                nc.scalar.activation(e_fm[:], ps_lg[:], AF.Exp, bias=gb2_sb[:])
                return ps_lg, e_fm

            # softmax tail, split so each PE piece can be emitted separately
            def blend_sum(m, e_fm):
                ps_s = pp.tile([1, NB], F32, name=f"ps_s_{m}", tag="ps",
                               bufs=4)
                nc.tensor.matmul(ps_s[:], ones8_sb[:], e_fm[:],
                                 start=True, stop=True)
                return ps_s

            def blend_ln(m, ps_s):
                # log-softmax: lnS on ACT (~0.5us) beats the 3.3us DVE
                # single-lane iterative-divide reciprocal
                lns = sp.tile([1, NB], F32R, name=f"lns_{m}")
                with nc.allow_low_precision(reason="f32r storage for lnS"):
                    nc.scalar.activation(lns[:], ps_s[:], AF.Ln)
                return lns

            def blend_norm(m, ps_lg, lns):
                # accumulate -lnS onto the logits (ones18_sb is all -1)
                nc.tensor.matmul(ps_lg[:], ones18_sb[:], lns[:],
                                 start=False, stop=True)

            def blend_exp(m, ps_lg):
                blend_fm = sp.tile([P, NB], BF16, name=f"blend_fm_{m}")
                nc.scalar.activation(blend_fm[:], ps_lg[:], AF.Exp,
                                     bias=gb2_sb[:])
                return blend_fm

            def bcast_one(m, blend_fm, p, out_list, copy_eng):
                # broadcast one blend row to 128 partitions via selector
                # matmul + ACT/DVE copy
                ps = pp.tile([128, NB], F32, name="ps_bc", tag="ps", bufs=4)
                nc.tensor.matmul(ps[:], selB_sb[:, p * 128:(p + 1) * 128],
                                 blend_fm[:], start=True, stop=True)
                bb = ap.tile([128, NB], BF16, name=f"bcb_{m}_{p}", bufs=1)
                if copy_eng == 0:
                    nc.scalar.copy(bb[:], ps[:])
                else:
                    nc.vector.tensor_copy(bb[:], ps[:])
                out_list.append(bb)

            # ------------- blended expert layers -------------
            def layer_acc(lname, eb_sb, blend_fm):
                """Allocate a layer's PSUM accumulators and emit the
                blended-bias matmuls (start=True)."""
                ps_l = []
                for o in range(NO):
                    ps = pp.tile([128, NB], F32, name=f"ps_{lname}_{o}",
                                 tag="acc", bufs=4)
                    nc.tensor.matmul(ps[:], eb_sb[:, o * 128:(o + 1) * 128],
                                     blend_fm[:], start=True, stop=False)
                    ps_l.append(ps)
                return ps_l

            def build_layer(m, lname, wsel, nk, xin, blendB, ps_l,
                            interleave=None, evac_scale=None):
                """One blended expert layer: out[o] = relu(sum_p W_p^T
                (x*b_p) + blend^T b).  interleave maps (k,p) chunk index ->
                fn emitting extra work between chunks."""
                ci = 0
                for k in range(nk):
                    for p in range(P):
                        if interleave and ci in interleave:
                            interleave[ci]()
                        ci += 1
                        xs = xsp.tile([128, NB], BF16, name="xs")
                        nc.vector.tensor_tensor(
                            xs[:], xin[k][:], blendB[p][:], op=ALU.mult)
                        for o in range(NO):
                            nc.tensor.matmul(
                                ps_l[o][:], wsel(p, k, o), xs[:],
                                start=False,
                                stop=(k == nk - 1 and p == P - 1))
                return _evac(lname, ps_l, evac_scale)

            def _evac(lname, ps_l, scale=None):
                xout = []
                for o in range(NO):
                    xt = ap.tile([128, NB], BF16, name=f"x_{lname[:2]}_{o}")
                    if scale is None:
                        if o % 2 == 0:
                            nc.scalar.activation(xt[:], ps_l[o][:], AF.Relu)
                        else:
                            nc.vector.tensor_scalar_max(xt[:], ps_l[o][:], 0.0)
                    else:
                        if o % 2 == 0:
                            nc.scalar.activation(xt[:], ps_l[o][:], AF.Relu,
                                                 scale=scale)
                        else:
                            nc.vector.tensor_scalar(xt[:], ps_l[o][:], scale,
                                                    0.0, op0=ALU.mult,
                                                    op1=ALU.max)
                    xout.append(xt)
                return xout

            def build_layer1(m, lname, xin, blendB, ps_l, interleave=None):
                """L1 with k-chunks 0,1 in fp8 DoubleRow (p-outer), chunks
                2,3 in bf16."""
                ci = 0
                for p in range(P):
                    if interleave and ci in interleave:
                        interleave[ci]()
                    ci += 1
                    xs8 = xsp.tile([128, 2 * NB], F8E4, name="xs8", bufs=3)
                    for k in range(2):
                        nc.vector.tensor_tensor(
                            xs8[:, k * NB:(k + 1) * NB], xin[k][:],
                            blendB[p][:], op=ALU.mult)
                    x3 = xs8[:].rearrange("p (two f) -> p two f", two=2)
                    for o in range(NO):
                        nc.tensor.matmul(ps_l[o][:], ew1f8_c(p, o), x3,
                                         start=False, stop=False,
                                         perf_mode=DR)
                    for k in range(2, KH):
                        if interleave and ci in interleave:
                            interleave[ci]()
                        ci += 1
                        xs = xsp.tile([128, NB], BF16, name="xs")
                        nc.vector.tensor_tensor(
                            xs[:], xin[k][:], blendB[p][:], op=ALU.mult)
                        for o in range(NO):
                            nc.tensor.matmul(
                                ps_l[o][:], ew1b_c(p, k, o), xs[:],
                                start=False,
                                stop=(k == KH - 1 and p == P - 1))
                return _evac(lname, ps_l, DESC)

            def build_l2_head(m, x2, c0=0, cw=NB):
                """Pair-stacked final-layer GEMMs + partition-sliced blend
                multiplies over columns [c0, c0+cw).  Returns tmul tiles;
                the fold matmuls are emitted later."""
                cs = slice(c0, c0 + cw)
                bb = bb_all[m]
                tmuls = []
                for j in range(4):
                    ps = pp.tile([128, cw], F32, name=f"ps2_{j}", tag="acc",
                                 bufs=4, padded_shape=[128, NB])
                    for k in range(KH):
                        nc.tensor.matmul(ps[:], ew2p_c(j, k), x2[k][:, cs],
                                         start=(k == 0), stop=(k == KH - 1))
                    tm = sp.tile([128, cw], BF16, name="tmul", bufs=3,
                                 padded_shape=[128, NB])
                    # experts 2j (rows 0:64) and 2j+1 (rows 64:128) scaled
                    # by their blend rows via slices of the bb tiles
                    nc.vector.tensor_tensor(tm[0:64, :], ps[0:64, :],
                                            bb[2 * j][0:64, cs],
                                            op=ALU.mult)
                    nc.vector.tensor_tensor(tm[64:128, :], ps[64:128, :],
                                            bb[2 * j + 1][64:128, cs],
                                            op=ALU.mult)
                    tmuls.append(tm)
                return tmuls

            def build_l2_fold(m, key, tmuls, blend_fm, js, c0=0, cw=NB):
                """Emit y-PSUM fold matmuls for pair indices js (0=bias)."""
                cs = slice(c0, c0 + cw)
                for j in js:
                    if j == 0:
                        y_ps[key] = pp.tile([ACT2, cw], F32,
                                            name=f"ps_y_{key}", tag="ps",
                                            bufs=4, padded_shape=[128, NB])
                        nc.tensor.matmul(y_ps[key][:], eb2_sb[:],
                                         blend_fm[:, cs],
                                         start=True, stop=False)
                    nc.tensor.matmul(y_ps[key][:], sel64_sb[:], tmuls[j][:],
                                     start=False, stop=(j == 3))

            def build_l2_tail(m, key, c0=0, cw=NB):
                yp = y_ps[key]
                mu = sp.tile([32, cw], F32, name="mu_sb", bufs=2,
                             padded_shape=[32, NB])
                nc.vector.tensor_copy(mu[:], yp[0:32, :])
                tls = sp.tile([32, cw], F32, name="tls", bufs=2,
                              padded_shape=[32, NB])
                nc.scalar.activation(tls[:], yp[32:ACT2, :], AF.Tanh)
                stdt = sp.tile([32, cw], F32, name="stdt", bufs=2,
                               padded_shape=[32, NB])
                nc.scalar.activation(stdt[:], tls[:], AF.Exp,
                                     scale=3.5, bias=neg15[0:32, :])
                gc = slice(m * NB + c0, m * NB + c0 + cw)
                nc.sync.dma_start(out_t[0:32, gc], mu[:])
                nc.sync.dma_start(out_t[32:ACT2, gc], stdt[:])

            # ---------------- schedule ----------------
            y_ps = {}
            bb_all = {0: [], 1: []}
            hold = {}

            lg0, e0 = build_gate(0)                  # gates m0

            def _m0_sum():
                hold["s0"] = blend_sum(0, e0)

            def _m0_tail():
                hold["lns0"] = blend_ln(0, hold["s0"])
                blend_norm(0, lg0, hold["lns0"])
                hold["bf0"] = blend_exp(0, lg0)

            # gates m1 cover the m0 softmax chain
            lg1, e1 = build_gate(1, after_g0=_m0_sum, after_g1=_m0_tail)
            bf0 = hold["bf0"]
            bb0 = bb_all[0]
            # first two m0 blend broadcasts up front (their copies land on
            # ACT/DVE ahead of m1's softmax tail) so L0a's inputs are ready
            # the moment gate m1 ends; the rest interleave into L0a chunks
            bcast_one(0, bf0, 0, bb0, 0)
            bcast_one(0, bf0, 1, bb0, 1)
            acc_l0a = layer_acc("l0a", eb0_sb, bf0)
            ps_s1 = blend_sum(1, e1)
            lns1 = blend_ln(1, ps_s1)
            blend_norm(1, lg1, lns1)
            bf1 = blend_exp(1, lg1)
            hold["bf1"] = bf1
            bb_m1 = bb_all[1]
            il0 = {}
            for i in range(6):
                il0[i + 1] = (lambda p=i + 2, e=i % 2: bcast_one(0, bf0, p,
                                                                bb0, e))
            for i in range(P):
                il0[8 + i] = (lambda p=i, e=i % 2: bcast_one(1, bf1, p,
                                                             bb_m1, e))

            obs0 = [obs_c(k, 0) for k in range(KO)]
            obs1 = [obs_c(k, 1) for k in range(KO)]
            x1_0 = build_layer(0, "l0a", ew0_c, KO, obs0, bb0, acc_l0a,
                               interleave=il0,
                               evac_scale=XSC if USE_FP8_L1 else None)
            acc = layer_acc("l0b", eb0_sb, hold["bf1"])
            x1_1 = build_layer(1, "l0b", ew0_c, KO, obs1, bb_m1, acc,
                               evac_scale=XSC if USE_FP8_L1 else None)
            acc = layer_acc("l1a", eb1_sb, bf0)
            if USE_FP8_L1:
                x2_0 = build_layer1(0, "l1a", x1_0, bb0, acc)
            else:
                x2_0 = build_layer(0, "l1a", ew1_c, KH, x1_0, bb0, acc)

            tm0 = build_l2_head(0, x2_0)             # L2 m0 GEMMs
            build_l2_fold(0, "m0", tm0, bf0, [0, 1, 2])

            il1 = {2: (lambda: build_l2_fold(0, "m0", tm0, bf0, [3])),
                   4: (lambda: build_l2_tail(0, "m0"))}
            acc = layer_acc("l1b", eb1_sb, hold["bf1"])
            if USE_FP8_L1:
                x2_1 = build_layer1(1, "l1b", x1_1, bb_m1, acc,
                                    interleave=il1)
            else:
                x2_1 = build_layer(1, "l1b", ew1_c, KH, x1_1, bb_m1, acc,
                                   interleave=il1)

            # m1 final layer in two column chunks so the tmul/fold/tail
            # chain of chunk A hides under chunk B's GEMMs
            HW = NB // 2
            tmA = build_l2_head(1, x2_1, 0, HW)
            tmB = build_l2_head(1, x2_1, HW, HW)
            build_l2_fold(1, "m1a", tmA, hold["bf1"], [0, 1, 2, 3], 0, HW)
            build_l2_tail(1, "m1a", 0, HW)
            build_l2_fold(1, "m1b", tmB, hold["bf1"], [0, 1, 2, 3], HW, HW)
            build_l2_tail(1, "m1b", HW, HW)

    _split_multi_waits(nc)
    return nc


_NC_CACHE = None


def _get_program():
    global _NC_CACHE
    if _NC_CACHE is None:
        _NC_CACHE = _build_program()
    return _NC_CACHE


def _prep_core_inputs(inputs):
    import ml_dtypes
    f32 = np.float32
    bf16 = ml_dtypes.bfloat16
    f8 = ml_dtypes.float8_e4m3
    obs = np.ascontiguousarray(inputs["obs"], dtype=f32)
    gw0 = np.asarray(inputs["gw0"], f32)
    gb0 = np.asarray(inputs["gb0"], f32)
    gw1 = np.asarray(inputs["gw1"], f32)
    gb1 = np.asarray(inputs["gb1"], f32)
    gw2 = np.asarray(inputs["gw2"], f32)
    gb2 = np.asarray(inputs["gb2"], f32)
    ew0 = np.asarray(inputs["ew0"], f32)
    eb0 = np.asarray(inputs["eb0"], f32)
    ew1 = np.asarray(inputs["ew1"], f32)
    eb1 = np.asarray(inputs["eb1"], f32)
    ew2 = np.asarray(inputs["ew2"], f32)
    eb2 = np.asarray(inputs["eb2"], f32)

    def chunk_cols(a, nk):
        # [nk*128, W] -> [128, nk*W] with chunk k at cols [k*W:(k+1)*W]
        return np.concatenate([a[k * 128:(k + 1) * 128, :]
                               for k in range(nk)], axis=1)

    obs_tf = np.ascontiguousarray(obs.T)                      # [OBS, B] f32
    obs_bt = obs_tf.astype(bf16)
    gw0_w = np.ascontiguousarray(chunk_cols(gw0.T, KO).astype(bf16))
    gw1_w = np.ascontiguousarray(chunk_cols(gw1.T, KH).astype(bf16))
    gw2_w = np.ascontiguousarray(chunk_cols(gw2.T, KH).astype(bf16))
    gb0_r = np.ascontiguousarray(gb0.reshape(NO, 128).T)      # [128, NO]
    gb1_r = np.ascontiguousarray(gb1.reshape(NO, 128).T)
    gb2_c = gb2.reshape(P, 1).astype(f32)
    # ew packed [128, nk*P*HID], chunk (k,p) at col (k*P+p)*HID
    ew0_t = ew0.transpose(0, 2, 1)                            # [P, OBS, HID]
    ew1_t = ew1.transpose(0, 2, 1)                            # [P, HID, HID]
    ew0_w = np.concatenate(
        [ew0_t[p, k * 128:(k + 1) * 128, :]
         for k in range(KO) for p in range(P)], axis=1).astype(bf16)
    if USE_FP8_L1:
        ew1s = ew1_t * WSC
        ew1f8_w = np.concatenate(
            [ew1s[p, k * 128:(k + 1) * 128, o * 128:(o + 1) * 128]
             for p in range(P) for o in range(NO) for k in range(2)],
            axis=1)
        ew1f8_w = np.clip(ew1f8_w, -240.0, 240.0).astype(f8)
        ew1b_w = np.concatenate(
            [ew1s[p, k * 128:(k + 1) * 128, :]
             for k in range(2, KH) for p in range(P)], axis=1).astype(bf16)
        eb1_m = (eb1 * (WSC * XSC)).astype(bf16)
    else:
        ew1_w = np.concatenate(
            [ew1_t[p, k * 128:(k + 1) * 128, :]
             for k in range(KH) for p in range(P)], axis=1).astype(bf16)
        eb1_m = eb1.astype(bf16)
    ew2_t = ew2.transpose(0, 2, 1)                            # [P, HID, 64]
    ew2p = [np.concatenate([ew2_t[2 * j], ew2_t[2 * j + 1]], axis=1)
            for j in range(4)]                                # [HID, 128] x4
    ew2p_w = np.concatenate(
        [ew2p[j][k * 128:(k + 1) * 128, :]
         for j in range(4) for k in range(KH)], axis=1).astype(bf16)
    ones8 = np.ones((P, 1), bf16)
    ones18 = -np.ones((1, P), f32)          # negated: accumulates -lnS
    selB = np.zeros((P, P * 128), bf16)
    for p in range(P):
        selB[p, p * 128:(p + 1) * 128] = 1.0
    sel64 = np.zeros((128, ACT2), bf16)
    for r in range(128):
        sel64[r, r % ACT2] = 1.0

    shared = {
        "gw0_w": gw0_w, "gw1_w": gw1_w, "gw2_w": gw2_w,
        "gb0_r": gb0_r, "gb1_r": gb1_r, "gb2_c": gb2_c,
        "ew0_w": np.ascontiguousarray(ew0_w),
        "ew2p_w": np.ascontiguousarray(ew2p_w),
        "eb0_m": eb0.astype(bf16), "eb1_m": eb1_m,
        "eb2_m": eb2.astype(bf16),
        "ones8": ones8, "ones18": ones18, "selB": selB,
        "sel64": sel64,
    }
    if USE_FP8_L1:
        shared["ew1f8_w"] = np.ascontiguousarray(ew1f8_w)
        shared["ew1b_w"] = np.ascontiguousarray(ew1b_w)
    else:
        shared["ew1_w"] = np.ascontiguousarray(ew1_w)
    in_maps = []
    for c in range(NCORES):
        im = dict(shared)
        oc = obs_bt[:, c * BL:(c + 1) * BL]                   # [OBS, BL]
        im["obs_w"] = np.ascontiguousarray(
            np.concatenate([oc[k * 128:(k + 1) * 128, m * NB:(m + 1) * NB]
                            for m in range(NMACRO) for k in range(KO)],
                           axis=1))
        in_maps.append(im)
    return in_maps


def kernel(**inputs) -> np.ndarray:
    import time

    from concourse.bass_utils import run_bass_kernel_spmd

    nc = _get_program()
    in_maps = _prep_core_inputs(inputs)
    res = None
    last_err = None
    # a freshly-compiled NEFF occasionally hits a transient
    # NRT_EXEC_UNIT_UNRECOVERABLE on its first execution; a retry succeeds
    for attempt in range(3):
        try:
            res = run_bass_kernel_spmd(nc, in_maps, core_ids=list(range(NCORES)))
            break
        except Exception as e:  # noqa: BLE001
            last_err = e
            time.sleep(2.0)
    if res is None:
        raise last_err
    out = np.concatenate(
        [res.results[c]["out_t"].T for c in range(NCORES)], axis=0)
    return np.ascontiguousarray(out, dtype=np.float32)


# revision 10
# speedup vs baseline: 1.2608x; 1.0048x over previous
"""Trainium2 Bass kernel for nn_DiagGaussianActor (MoE-routing actor MLP).

Data-parallel over 8 NeuronCores: batch 8192 is split into 8 shards of
1024; gate + expert weights are replicated. Per core, the blended-expert
MLP runs with all GEMMs on the tensor engine (fp32 PSUM accumulation):

  - activations kept feature-major [feat, batch]; N=512 matmuls
  - the gate MLP runs in bf16 (FWL hides LDWEIGHTS under the matmuls)
  - per-layer expert blending folded into the GEMM: inputs are scaled by
    broadcast blend tiles (xs_p = x * blend[p,:]) and all 8 experts
    accumulate into one PSUM bank; the per-sample blended bias is added
    with a K=8 matmul against blend_fm [8, batch]
  - the first half (k-chunks 0,1) of the middle expert layer runs in
    fp8e4m3 DoubleRow mode (2 k-planes per matmul, ~1.7x PE throughput);
    weights are pre-scaled x512 host-side, activations x32 on the L0
    evacuation, and the PSUM descaled 2^-14 on the L1 evacuation
  - the final layer is pair-stacked feature-major; experts are combined
    by partition-sliced blend multiplies (DVE, reusing the bb broadcast
    tiles) and a [I64;I64] selector matmul that folds the two 64-row
    halves straight into a PSUM bank pre-loaded with the blended bias
  - weights are repacked host-side into [128, W] tensors with multi-KB
    contiguous per-partition lines and DMA'd in consumption order over
    several queues
  - the two 512-column macro-batches are software-pipelined: macro 1's
    gate matmuls cover macro 0's softmax/blend latency, the blend
    broadcasts for m0 are emitted ahead of m1's softmax tail so L0a can
    start the moment gate m1 ends (no HAM re-throttle), and macro 0's
    final layer is interleaved into macro 1's hidden-layer matmuls
"""
import sys

sys.path.insert(0, "/opt/trn_rl_repo")

import numpy as np

import concourse.bass as bass
import concourse.mybir as mybir
import concourse.tile as tile
from concourse.vector_clock import ScopedClock, VectorClock

F32 = mybir.dt.float32
F32R = mybir.dt.float32r
BF16 = mybir.dt.bfloat16
F8E4 = mybir.dt.float8e4
AF = mybir.ActivationFunctionType
ALU = mybir.AluOpType
DR = mybir.MatmulPerfMode.DoubleRow

B = 8192
OBS = 256
ACT2 = 64  # 2 * action_dim
HID = 512
P = 8
NCORES = 8
BL = B // NCORES          # batch per core = 1024
NB = 512                  # macro-batch (free-dim) size
NMACRO = BL // NB         # 2
KO = OBS // 128           # 2 obs k-chunks
KH = HID // 128           # 4 hidden k-chunks
NO = HID // 128           # 4 output chunks for HID layers
NWARM = 68                # HAM warm-up matmuls (N=64)

USE_FP8_L1 = True         # k-chunks 0,1 of L1 in fp8 DoubleRow
XSC = 32.0                # activation scale applied on the L0 evac
WSC = 512.0               # L1 weight scale (both halves)
DESC = 1.0 / (XSC * WSC)  # L1 evac descale


class _SplitDrainTileContext(tile.TileContext):
    """The walrus build in this container accepts very few sync-wait
    commands per instruction; the stock kernel-tail drain carries one wait
    per logical proc and fails codegen. Emit one SP nop per proc instead."""

    def _drain_and_barrier(self, tick_clock, wait_clock):
        gc = tick_clock.global_clock
        vec = list(gc)
        n = len(vec)
        for i, t in enumerate(vec):
            if t <= 0:
                continue
            sub = VectorClock([vec[j] if j == i else 0 for j in range(n)])
            nop = self.nc.sync.nop(nofuse=True)
            wait_clock.add_sem_waits(nop.ins, ScopedClock({None: sub}))
        self.nc.sync.drain()
        self.nc.all_engine_barrier()
        assert self.sems is not None
        popped = self.nc._tile_sem_poison_stack.pop()
        assert popped is self._sem_poison
        self.nc.clear_and_free_semaphores(list(self.sems.allocated().values()))
        self.nc.all_engine_barrier()


def _split_multi_waits(nc):
    """Hoist all but one sync-wait of each instruction onto NoOps on the
    same engine placed immediately before it (same queue => order kept)."""
    for f in nc.m.functions:
        for bb in f.blocks:
            new_insts = []
            for inst in bb.instructions:
                si = getattr(inst, "sync_info", None)
                ow = list(getattr(si, "on_wait", None) or [])
                if len(ow) > 1:
                    for w in ow[:-1]:
                        nop = mybir.InstNoOp(
                            name=f"I-waitsplit-{nc.next_id()}", ins=[], outs=[]
                        )
                        nop.engine = inst.engine
                        nop.sync_info = mybir.SyncInfo(on_wait=[w], on_update=[])
                        new_insts.append(nop)
                    si.on_wait = [ow[-1]]
                new_insts.append(inst)
            bb.instructions[:] = new_insts


def _build_program():
    nc = bass.Bass("TRN2", target_bir_lowering=False, debug=False)

    def din(name, shape, dtype=BF16):
        return nc.dram_tensor(name, shape, dtype, kind="ExternalInput").ap()

    # packed [128, W] weight tensors; col layout noted per tensor
    obs_w = din("obs_w", [128, KO * BL])          # chunk (m,k) at (m*KO+k)*NB
    gw0_w = din("gw0_w", [128, KO * HID])         # chunk k at cols k*HID
    gw1_w = din("gw1_w", [128, KH * HID])
    gw2_w = din("gw2_w", [128, KH * P])           # chunk k at cols k*P
    gb0_r = din("gb0_r", [128, NO], F32)          # col o = gb0[128o:128(o+1)]
    gb1_r = din("gb1_r", [128, NO], F32)
    gb2_c = din("gb2_c", [P, 1], F32)
    ew0_w = din("ew0_w", [128, KO * P * HID])     # chunk (k,p) at (k*P+p)*HID
    if USE_FP8_L1:
        # pair (k0,k1) per (p,o): 256 cols at (p*NO+o)*256, scaled x512
        ew1f8_w = din("ew1f8_w", [128, P * NO * 256], F8E4)
        # chunks k2,k3: chunk (k,p) at ((k-2)*P+p)*HID, scaled x512
        ew1b_w = din("ew1b_w", [128, 2 * P * HID])
    else:
        ew1_w = din("ew1_w", [128, KH * P * HID])
    ew2p_w = din("ew2p_w", [128, 4 * KH * 128])   # chunk (j,k) at (j*KH+k)*128
    eb0_m = din("eb0_m", [P, HID])
    eb1_m = din("eb1_m", [P, HID])
    eb2_m = din("eb2_m", [P, ACT2])
    ones8 = din("ones8", [P, 1], BF16)
    ones18 = din("ones18", [1, P], F32)
    selB = din("selB", [P, P * 128])              # selB[:, 128p:] = one-hot row p
    sel64 = din("sel64", [128, ACT2])             # [I64; I64]

    out_t = nc.dram_tensor("out_t", [ACT2, BL], F32, kind="ExternalOutput").ap()

    with _SplitDrainTileContext(nc) as tc:
        with tc.tile_pool(name="wp", bufs=1) as wp, \
             tc.tile_pool(name="ap", bufs=2) as ap, \
             tc.tile_pool(name="xsp", bufs=6) as xsp, \
             tc.tile_pool(name="sp", bufs=2) as sp, \
             tc.tile_pool(name="pp", bufs=8, space="PSUM") as pp:

            # ---- load weights / consts, in consumption order ----
            def wtile(name, shape, src, dtype=BF16, nsplit=1, engs=None):
                engs = engs or [nc.sync]
                t = wp.tile(shape, dtype, name=name)
                w = shape[1] // nsplit
                for s in range(nsplit):
                    engs[s % len(engs)].dma_start(
                        t[:, s * w:(s + 1) * w], src[:, s * w:(s + 1) * w])
                return t

            # gate-critical path: the first ~1 MB is spread over four DGE
            # queues so the gate can start as soon as possible; the bulk
            # expert weights alternate the sync/scalar queues.  GpSimd
            # SWDGE carries the tiny consts.
            obs_sb = wp.tile([128, KO * BL], BF16, name="obs_sb")
            gw0_sb = wp.tile([128, KO * HID], BF16, name="gw0_sb")
            gw1_sb = wp.tile([128, KH * HID], BF16, name="gw1_sb")
            nc.sync.dma_start(gw0_sb[:, 0:HID], gw0_w[:, 0:HID])
            nc.scalar.dma_start(obs_sb[:, 0:NB], obs_w[:, 0:NB])
            nc.gpsimd.dma_start(gw0_sb[:, HID:2 * HID],
                                gw0_w[:, HID:2 * HID])
            nc.scalar.dma_start(obs_sb[:, NB:2 * NB], obs_w[:, NB:2 * NB])
            nc.sync.dma_start(gw1_sb[:, 0:HID], gw1_w[:, 0:HID])
            nc.gpsimd.dma_start(gw1_sb[:, HID:2 * HID],
                                gw1_w[:, HID:2 * HID])
            nc.sync.dma_start(gw1_sb[:, 2 * HID:3 * HID],
                              gw1_w[:, 2 * HID:3 * HID])
            nc.scalar.dma_start(gw1_sb[:, 3 * HID:4 * HID],
                                gw1_w[:, 3 * HID:4 * HID])
            nc.sync.dma_start(obs_sb[:, KO * NB:KO * BL],
                              obs_w[:, KO * NB:KO * BL])
            gw2_sb = wtile("gw2_sb", [128, KH * P], gw2_w, engs=[nc.gpsimd])
            gb0_sb = wtile("gb0_sb", [128, NO], gb0_r, F32, engs=[nc.gpsimd])
            gb1_sb = wtile("gb1_sb", [128, NO], gb1_r, F32, engs=[nc.gpsimd])
            gb2_sb = wtile("gb2_sb", [P, 1], gb2_c, F32, engs=[nc.gpsimd])
            ones8_sb = wtile("ones8_sb", [P, 1], ones8, engs=[nc.gpsimd])
            ones18_sb = wtile("ones18_sb", [1, P], ones18.bitcast(F32R), F32R,
                              engs=[nc.gpsimd])
            eb0_sb = wtile("eb0_sb", [P, HID], eb0_m, engs=[nc.gpsimd])
            selB_sb = wtile("selB_sb", [P, P * 128], selB, engs=[nc.gpsimd])
            eb1_sb = wtile("eb1_sb", [P, HID], eb1_m, engs=[nc.gpsimd])
            # pre-warm the ACT LUTs (first use of each table costs ~1.3us)
            # while the bulk DMAs stream; keeps table loads off the
            # softmax/evac critical path
            twarm = wp.tile([128, 1], F32, name="twarm")
            nc.vector.memset(twarm[:], 1.0)
            for af in (AF.Exp, AF.Ln, AF.Relu, AF.Tanh):
                nc.scalar.activation(twarm[:], twarm[:], af)
            nc.scalar.copy(twarm[:], twarm[:])
            ew0_sb = wtile("ew0_sb", [128, KO * P * HID], ew0_w, nsplit=2,
                           engs=[nc.sync, nc.scalar])
            if USE_FP8_L1:
                ew1f8_sb = wtile("ew1f8_sb", [128, P * NO * 256], ew1f8_w,
                                 F8E4, nsplit=2, engs=[nc.sync, nc.scalar])
                ew1b_sb = wtile("ew1b_sb", [128, 2 * P * HID], ew1b_w,
                                nsplit=2, engs=[nc.sync, nc.scalar])
            else:
                ew1_sb = wtile("ew1_sb", [128, KH * P * HID], ew1_w, nsplit=4,
                               engs=[nc.sync, nc.scalar])
            ew2p_sb = wtile("ew2p_sb", [128, 4 * KH * 128], ew2p_w)
            sel64_sb = wtile("sel64_sb", [128, ACT2], sel64,
                             engs=[nc.gpsimd])
            eb2_sb = wtile("eb2_sb", [P, ACT2], eb2_m, engs=[nc.gpsimd])

            def ew0_c(p, k, o):
                c = (k * P + p) * HID + o * 128
                return ew0_sb[:, c:c + 128]

            def ew1b_c(p, k, o):
                # k in {2, 3}
                c = ((k - 2) * P + p) * HID + o * 128
                return ew1b_sb[:, c:c + 128]

            def ew1f8_c(p, o):
                c = (p * NO + o) * 256
                return ew1f8_sb[:, c:c + 256].rearrange(
                    "p (two f) -> p two f", two=2)

            def ew2p_c(j, k):
                c = (j * KH + k) * 128
                return ew2p_sb[:, c:c + 128]

            def obs_c(k, m):
                c = (m * KO + k) * NB
                return obs_sb[:, c:c + NB]

            neg15 = wp.tile([128, 1], F32, name="neg15")
            nc.vector.memset(neg15[:], -1.5)
            # HAM warm-up: keep the PE busy while the first DMAs land
            warm = wp.tile([128, 128], BF16, name="warm")
            nc.vector.memset(warm[:], 1.0)
            ps_w = pp.tile([128, 64], F32, name="ps_warm", tag="ps", bufs=4,
                           padded_shape=[128, NB])
            for _ in range(NWARM):
                nc.tensor.matmul(ps_w[:], warm[:], warm[:, 0:64],
                                 start=True, stop=True)

            # ---------------- gate network ----------------
            def gate_layer(m, lid, wsb, nk, xin, gb_sb):
                # k-outer emission: first matmuls need only the k=0 weight
                # chunk, so compute can start on partially-arrived DMAs
                pss = [pp.tile([128, NB], F32, name=f"ps_g{lid}_{o}",
                               tag="acc", bufs=4) for o in range(NO)]
                for k in range(nk):
                    for o in range(NO):
                        nc.tensor.matmul(
                            pss[o][:],
                            wsb[:, k * HID + o * 128:k * HID + (o + 1) * 128],
                            xin[k][:], start=(k == 0), stop=(k == nk - 1))
                xg = []
                for o in range(NO):
                    xt = ap.tile([128, NB], BF16, name=f"xg{lid}_{m}_{o}",
                                 bufs=1)
                    nc.vector.tensor_scalar(xt[:], pss[o][:],
                                            gb_sb[:, o:o + 1], 0.0,
                                            op0=ALU.add, op1=ALU.max)
                    xg.append(xt)
                return xg

            def build_gate(m, after_g0=None, after_g1=None):
                obs_m = [obs_c(k, m) for k in range(KO)]
                xg0 = gate_layer(m, 0, gw0_sb, KO, obs_m, gb0_sb)
                if after_g0 is not None:
                    after_g0()
                xg1 = gate_layer(m, 1, gw1_sb, KH, xg0, gb1_sb)
                if after_g1 is not None:
                    after_g1()
                ps_lg = pp.tile([P, NB], F32, name=f"ps_lg_{m}", tag="ps",
                                bufs=4)
                for k in range(KH):
                    nc.tensor.matmul(ps_lg[:], gw2_sb[:, k * P:(k + 1) * P],
                                     xg1[k][:], start=(k == 0), stop=False)
                e_fm = sp.tile([P, NB], BF16, name=f"e_fm_{m}")
                nc.scalar.activation(e_fm[:], ps_lg[:], AF.Exp, bias=gb2_sb[:])
                return ps_lg, e_fm

            # softmax tail, split so each PE piece can be emitted separately
            def blend_sum(m, e_fm):
                ps_s = pp.tile([1, NB], F32, name=f"ps_s_{m}", tag="ps",
                               bufs=4)
                nc.tensor.matmul(ps_s[:], ones8_sb[:], e_fm[:],
                                 start=True, stop=True)
                return ps_s

            def blend_ln(m, ps_s):
                # log-softmax: lnS on ACT (~0.5us) beats the 3.3us DVE
                # single-lane iterative-divide reciprocal
                lns = sp.tile([1, NB], F32R, name=f"lns_{m}")
                with nc.allow_low_precision(reason="f32r storage for lnS"):
                    nc.scalar.activation(lns[:], ps_s[:], AF.Ln)
                return lns

            def blend_norm(m, ps_lg, lns):
                # accumulate -lnS onto the logits (ones18_sb is all -1)
                nc.tensor.matmul(ps_lg[:], ones18_sb[:], lns[:],
                                 start=False, stop=True)

            def blend_exp(m, ps_lg):
                blend_fm = sp.tile([P, NB], BF16, name=f"blend_fm_{m}")
                nc.scalar.activation(blend_fm[:], ps_lg[:], AF.Exp,
                                     bias=gb2_sb[:])
                return blend_fm

            def bcast_one(m, blend_fm, p, out_list, copy_eng):
                # broadcast one blend row to 128 partitions via selector
                # matmul + ACT/DVE copy
                ps = pp.tile([128, NB], F32, name="ps_bc", tag="ps", bufs=4)
                nc.tensor.matmul(ps[:], selB_sb[:, p * 128:(p + 1) * 128],
                                 blend_fm[:], start=True, stop=True)
                bb = ap.tile([128, NB], BF16, name=f"bcb_{m}_{p}", bufs=1)
                if copy_eng == 0:
                    nc.scalar.copy(bb[:], ps[:])
                else:
                    nc.vector.tensor_copy(bb[:], ps[:])
                out_list.append(bb)

            # ------------- blended expert layers -------------
            def layer_acc(lname, eb_sb, blend_fm):
                """Allocate a layer's PSUM accumulators and emit the
                blended-bias matmuls (start=True)."""
                ps_l = []
                for o in range(NO):
                    ps = pp.tile([128, NB], F32, name=f"ps_{lname}_{o}",
                                 tag="acc", bufs=4)
                    nc.tensor.matmul(ps[:], eb_sb[:, o * 128:(o + 1) * 128],
                                     blend_fm[:], start=True, stop=False)
                    ps_l.append(ps)
                return ps_l

            def build_layer(m, lname, wsel, nk, xin, blendB, ps_l,
                            interleave=None, evac_scale=None):
                """One blended expert layer: out[o] = relu(sum_p W_p^T
                (x*b_p) + blend^T b).  interleave maps (k,p) chunk index ->
                fn emitting extra work between chunks."""
                ci = 0
                for k in range(nk):
                    for p in range(P):
                        if interleave and ci in interleave:
                            interleave[ci]()
                        ci += 1
                        xs = xsp.tile([128, NB], BF16, name="xs")
                        nc.vector.tensor_tensor(
                            xs[:], xin[k][:], blendB[p][:], op=ALU.mult)
                        for o in range(NO):
                            nc.tensor.matmul(
                                ps_l[o][:], wsel(p, k, o), xs[:],
                                start=False,
                                stop=(k == nk - 1 and p == P - 1))
                return _evac(lname, ps_l, evac_scale)

            def _evac(lname, ps_l, scale=None):
                xout = []
                for o in range(NO):
                    xt = ap.tile([128, NB], BF16, name=f"x_{lname[:2]}_{o}")
                    if scale is None:
                        if o % 2 == 0:
                            nc.scalar.activation(xt[:], ps_l[o][:], AF.Relu)
                        else:
                            nc.vector.tensor_scalar_max(xt[:], ps_l[o][:], 0.0)
                    else:
                        if o % 2 == 0:
                            nc.scalar.activation(xt[:], ps_l[o][:], AF.Relu,
                                                 scale=scale)
                        else:
                            nc.vector.tensor_scalar(xt[:], ps_l[o][:], scale,
                                                    0.0, op0=ALU.mult,
                                                    op1=ALU.max)
                    xout.append(xt)
                return xout

            def build_layer1(m, lname, xin, blendB, ps_l, interleave=None):
                """L1 with k-chunks 0,1 in fp8 DoubleRow (p-outer), chunks
                2,3 in bf16."""
                ci = 0
                for p in range(P):
                    if interleave and ci in interleave:
                        interleave[ci]()
                    ci += 1
                    xs8 = xsp.tile([128, 2 * NB], F8E4, name="xs8", bufs=3)
                    for k in range(2):
                        nc.vector.tensor_tensor(
                            xs8[:, k * NB:(k + 1) * NB], xin[k][:],
                            blendB[p][:], op=ALU.mult)
                    x3 = xs8[:].rearrange("p (two f) -> p two f", two=2)
                    for o in range(NO):
                        nc.tensor.matmul(ps_l[o][:], ew1f8_c(p, o), x3,
                                         start=False, stop=False,
                                         perf_mode=DR)
                    for k in range(2, KH):
                        if interleave and ci in interleave:
                            interleave[ci]()
                        ci += 1
                        xs = xsp.tile([128, NB], BF16, name="xs")
                        nc.vector.tensor_tensor(
                            xs[:], xin[k][:], blendB[p][:], op=ALU.mult)
                        for o in range(NO):
                            nc.tensor.matmul(
                                ps_l[o][:], ew1b_c(p, k, o), xs[:],
                                start=False,
                                stop=(k == KH - 1 and p == P - 1))
                return _evac(lname, ps_l, DESC)

            def build_l2_head(m, x2, c0=0, cw=NB):
                """Pair-stacked final-layer GEMMs + partition-sliced blend
                multiplies over columns [c0, c0+cw).  Returns tmul tiles;
                the fold matmuls are emitted later."""
                cs = slice(c0, c0 + cw)
                bb = bb_all[m]
                tmuls = []
                for j in range(4):
                    ps = pp.tile([128, cw], F32, name=f"ps2_{j}", tag="acc",
                                 bufs=4, padded_shape=[128, NB])
                    for k in range(KH):
                        nc.tensor.matmul(ps[:], ew2p_c(j, k), x2[k][:, cs],
                                         start=(k == 0), stop=(k == KH - 1))
                    tm = sp.tile([128, cw], BF16, name="tmul", bufs=3,
                                 padded_shape=[128, NB])
                    # experts 2j (rows 0:64) and 2j+1 (rows 64:128) scaled
                    # by their blend rows via slices of the bb tiles
                    nc.vector.tensor_tensor(tm[0:64, :], ps[0:64, :],
                                            bb[2 * j][0:64, cs],
                                            op=ALU.mult)
                    nc.vector.tensor_tensor(tm[64:128, :], ps[64:128, :],
                                            bb[2 * j + 1][64:128, cs],
                                            op=ALU.mult)
                    tmuls.append(tm)
                return tmuls

            def build_l2_fold(m, key, tmuls, blend_fm, js, c0=0, cw=NB):
                """Emit y-PSUM fold matmuls for pair indices js (0=bias)."""
                cs = slice(c0, c0 + cw)
                for j in js:
                    if j == 0:
                        y_ps[key] = pp.tile([ACT2, cw], F32,
                                            name=f"ps_y_{key}", tag="ps",
                                            bufs=4, padded_shape=[128, NB])
                        nc.tensor.matmul(y_ps[key][:], eb2_sb[:],
                                         blend_fm[:, cs],
                                         start=True, stop=False)
                    nc.tensor.matmul(y_ps[key][:], sel64_sb[:], tmuls[j][:],
                                     start=False, stop=(j == 3))

            def build_l2_tail(m, key, c0=0, cw=NB):
                yp = y_ps[key]
                mu = sp.tile([32, cw], F32, name="mu_sb", bufs=2,
                             padded_shape=[32, NB])
                nc.vector.tensor_copy(mu[:], yp[0:32, :])
                tls = sp.tile([32, cw], F32, name="tls", bufs=2,
                              padded_shape=[32, NB])
                nc.scalar.activation(tls[:], yp[32:ACT2, :], AF.Tanh)
                stdt = sp.tile([32, cw], F32, name="stdt", bufs=2,
                               padded_shape=[32, NB])
                nc.scalar.activation(stdt[:], tls[:], AF.Exp,
                                     scale=3.5, bias=neg15[0:32, :])
                gc = slice(m * NB + c0, m * NB + c0 + cw)
                nc.sync.dma_start(out_t[0:32, gc], mu[:])
                nc.sync.dma_start(out_t[32:ACT2, gc], stdt[:])

            # ---------------- schedule ----------------
            y_ps = {}
            bb_all = {0: [], 1: []}
            hold = {}

            lg0, e0 = build_gate(0)                  # gates m0

            def _m0_sum():
                hold["s0"] = blend_sum(0, e0)

            def _m0_tail():
                hold["lns0"] = blend_ln(0, hold["s0"])
                blend_norm(0, lg0, hold["lns0"])
                hold["bf0"] = blend_exp(0, lg0)

            # gates m1 cover the m0 softmax chain
            lg1, e1 = build_gate(1, after_g0=_m0_sum, after_g1=_m0_tail)
            bf0 = hold["bf0"]
            bb0 = bb_all[0]
            # first two m0 blend broadcasts up front (their copies land on
            # ACT/DVE ahead of m1's softmax tail) so L0a's inputs are ready
            # the moment gate m1 ends; the rest interleave into L0a chunks
            bcast_one(0, bf0, 0, bb0, 0)
            bcast_one(0, bf0, 1, bb0, 1)
            acc_l0a = layer_acc("l0a", eb0_sb, bf0)
            ps_s1 = blend_sum(1, e1)
            lns1 = blend_ln(1, ps_s1)
            blend_norm(1, lg1, lns1)
            bf1 = blend_exp(1, lg1)
            hold["bf1"] = bf1
            bb_m1 = bb_all[1]
            il0 = {}
            for i in range(6):
                il0[i + 1] = (lambda p=i + 2, e=i % 2: bcast_one(0, bf0, p,
                                                                bb0, e))
            for i in range(P):
                il0[8 + i] = (lambda p=i, e=i % 2: bcast_one(1, bf1, p,
                                                             bb_m1, e))

            obs0 = [obs_c(k, 0) for k in range(KO)]
            obs1 = [obs_c(k, 1) for k in range(KO)]
            x1_0 = build_layer(0, "l0a", ew0_c, KO, obs0, bb0, acc_l0a,
                               interleave=il0,
                               evac_scale=XSC if USE_FP8_L1 else None)
            acc = layer_acc("l0b", eb0_sb, hold["bf1"])
            x1_1 = build_layer(1, "l0b", ew0_c, KO, obs1, bb_m1, acc,
                               evac_scale=XSC if USE_FP8_L1 else None)
            acc = layer_acc("l1a", eb1_sb, bf0)
            if USE_FP8_L1:
                x2_0 = build_layer1(0, "l1a", x1_0, bb0, acc)
            else:
                x2_0 = build_layer(0, "l1a", ew1_c, KH, x1_0, bb0, acc)

            tm0 = build_l2_head(0, x2_0)             # L2 m0 GEMMs
            build_l2_fold(0, "m0", tm0, bf0, [0, 1, 2])

            il1 = {2: (lambda: build_l2_fold(0, "m0", tm0, bf0, [3])),
                   4: (lambda: build_l2_tail(0, "m0"))}
            acc = layer_acc("l1b", eb1_sb, hold["bf1"])
            if USE_FP8_L1:
                x2_1 = build_layer1(1, "l1b", x1_1, bb_m1, acc,
                                    interleave=il1)
            else:
                x2_1 = build_layer(1, "l1b", ew1_c, KH, x1_1, bb_m1, acc,
                                   interleave=il1)

            # m1 final layer in two column chunks so the tmul/fold/tail
            # chain of chunk A hides under chunk B's GEMMs
            HW = NB // 2
            tmA = build_l2_head(1, x2_1, 0, HW)
            tmB = build_l2_head(1, x2_1, HW, HW)
            build_l2_fold(1, "m1a", tmA, hold["bf1"], [0, 1, 2, 3], 0, HW)
            build_l2_tail(1, "m1a", 0, HW)
            build_l2_fold(1, "m1b", tmB, hold["bf1"], [0, 1, 2, 3], HW, HW)
            build_l2_tail(1, "m1b", HW, HW)

    _split_multi_waits(nc)
    return nc


_NC_CACHE = None


def _get_program():
    global _NC_CACHE
    if _NC_CACHE is None:
        _NC_CACHE = _build_program()
    return _NC_CACHE


def _prep_core_inputs(inputs):
    import ml_dtypes
    f32 = np.float32
    bf16 = ml_dtypes.bfloat16
    f8 = ml_dtypes.float8_e4m3
    obs = np.ascontiguousarray(inputs["obs"], dtype=f32)
    gw0 = np.asarray(inputs["gw0"], f32)
    gb0 = np.asarray(inputs["gb0"], f32)
    gw1 = np.asarray(inputs["gw1"], f32)
    gb1 = np.asarray(inputs["gb1"], f32)
    gw2 = np.asarray(inputs["gw2"], f32)
    gb2 = np.asarray(inputs["gb2"], f32)
    ew0 = np.asarray(inputs["ew0"], f32)
    eb0 = np.asarray(inputs["eb0"], f32)
    ew1 = np.asarray(inputs["ew1"], f32)
    eb1 = np.asarray(inputs["eb1"], f32)
    ew2 = np.asarray(inputs["ew2"], f32)
    eb2 = np.asarray(inputs["eb2"], f32)

    def chunk_cols(a, nk):
        # [nk*128, W] -> [128, nk*W] with chunk k at cols [k*W:(k+1)*W]
        return np.concatenate([a[k * 128:(k + 1) * 128, :]
                               for k in range(nk)], axis=1)

    obs_tf = np.ascontiguousarray(obs.T)                      # [OBS, B] f32
    obs_bt = obs_tf.astype(bf16)
    gw0_w = np.ascontiguousarray(chunk_cols(gw0.T, KO).astype(bf16))
    gw1_w = np.ascontiguousarray(chunk_cols(gw1.T, KH).astype(bf16))
    gw2_w = np.ascontiguousarray(chunk_cols(gw2.T, KH).astype(bf16))
    gb0_r = np.ascontiguousarray(gb0.reshape(NO, 128).T)      # [128, NO]
    gb1_r = np.ascontiguousarray(gb1.reshape(NO, 128).T)
    gb2_c = gb2.reshape(P, 1).astype(f32)
    # ew packed [128, nk*P*HID], chunk (k,p) at col (k*P+p)*HID
    ew0_t = ew0.transpose(0, 2, 1)                            # [P, OBS, HID]
    ew1_t = ew1.transpose(0, 2, 1)                            # [P, HID, HID]
    ew0_w = np.concatenate(
        [ew0_t[p, k * 128:(k + 1) * 128, :]
         for k in range(KO) for p in range(P)], axis=1).astype(bf16)
    if USE_FP8_L1:
        ew1s = ew1_t * WSC
        ew1f8_w = np.concatenate(
            [ew1s[p, k * 128:(k + 1) * 128, o * 128:(o + 1) * 128]
             for p in range(P) for o in range(NO) for k in range(2)],
            axis=1)
        ew1f8_w = np.clip(ew1f8_w, -240.0, 240.0).astype(f8)
        ew1b_w = np.concatenate(
            [ew1s[p, k * 128:(k + 1) * 128, :]
             for k in range(2, KH) for p in range(P)], axis=1).astype(bf16)
        eb1_m = (eb1 * (WSC * XSC)).astype(bf16)
    else:
        ew1_w = np.concatenate(
            [ew1_t[p, k * 128:(k + 1) * 128, :]
             for k in range(KH) for p in range(P)], axis=1).astype(bf16)
        eb1_m = eb1.astype(bf16)
    ew2_t = ew2.transpose(0, 2, 1)                            # [P, HID, 64]
    ew2p = [np.concatenate([ew2_t[2 * j], ew2_t[2 * j + 1]], axis=1)
            for j in range(4)]                                # [HID, 128] x4
    ew2p_w = np.concatenate(
        [ew2p[j][k * 128:(k + 1) * 128, :]
         for j in range(4) for k in range(KH)], axis=1).astype(bf16)
    ones8 = np.ones((P, 1), bf16)
    ones18 = -np.ones((1, P), f32)          # negated: accumulates -lnS
    selB = np.zeros((P, P * 128), bf16)
    for p in range(P):
        selB[p, p * 128:(p + 1) * 128] = 1.0
    sel64 = np.zeros((128, ACT2), bf16)
    for r in range(128):
        sel64[r, r % ACT2] = 1.0

    shared = {
        "gw0_w": gw0_w, "gw1_w": gw1_w, "gw2_w": gw2_w,
        "gb0_r": gb0_r, "gb1_r": gb1_r, "gb2_c": gb2_c,
        "ew0_w": np.ascontiguousarray(ew0_w),
        "ew2p_w": np.ascontiguousarray(ew2p_w),
        "eb0_m": eb0.astype(bf16), "eb1_m": eb1_m,
        "eb2_m": eb2.astype(bf16),
        "ones8": ones8, "ones18": ones18, "selB": selB,
        "sel64": sel64,
    }
    if USE_FP8_L1:
        shared["ew1f8_w"] = np.ascontiguousarray(ew1f8_w)
        shared["ew1b_w"] = np.ascontiguousarray(ew1b_w)
    else:
        shared["ew1_w"] = np.ascontiguousarray(ew1_w)
    in_maps = []
    for c in range(NCORES):
        im = dict(shared)
        oc = obs_bt[:, c * BL:(c + 1) * BL]                   # [OBS, BL]
        im["obs_w"] = np.ascontiguousarray(
            np.concatenate([oc[k * 128:(k + 1) * 128, m * NB:(m + 1) * NB]
                            for m in range(NMACRO) for k in range(KO)],
                           axis=1))
        in_maps.append(im)
    return in_maps


def kernel(**inputs) -> np.ndarray:
    import time

    from concourse.bass_utils import run_bass_kernel_spmd

    nc = _get_program()
    in_maps = _prep_core_inputs(inputs)
    res = None
    last_err = None
    # a freshly-compiled NEFF occasionally hits a transient
    # NRT_EXEC_UNIT_UNRECOVERABLE on its first execution; a retry succeeds
    for attempt in range(3):
        try:
            res = run_bass_kernel_spmd(nc, in_maps, core_ids=list(range(NCORES)))
            break
        except Exception as e:  # noqa: BLE001
            last_err = e
            time.sleep(2.0)
    if res is None:
        raise last_err
    out = np.concatenate(
        [res.results[c]["out_t"].T for c in range(NCORES)], axis=0)
    return np.ascontiguousarray(out, dtype=np.float32)


# revision 20
# speedup vs baseline: 1.3951x; 1.1065x over previous
"""Trainium2 Bass kernel for nn_DiagGaussianActor (MoE-routing actor MLP).

Data-parallel over 8 NeuronCores: batch 8192 is split into 8 shards of
1024; gate + expert weights are replicated. Per core, the blended-expert
MLP runs with all GEMMs on the tensor engine (fp32 PSUM accumulation):

  - activations kept feature-major [feat, batch]; N=512 matmuls
  - the gate MLP runs in bf16 (FWL hides LDWEIGHTS under the matmuls)
  - per-layer expert blending folded into the GEMM: inputs are scaled by
    broadcast blend tiles (xs_p = x * blend[p,:]) and all 8 experts
    accumulate into one PSUM bank; the per-sample blended bias is added
    with a K=8 matmul against blend_fm [8, batch]
  - the first half (k-chunks 0,1) of the middle expert layer runs in
    fp8e4m3 DoubleRow mode (2 k-planes per matmul, ~1.7x PE throughput);
    weights are pre-scaled x512 host-side, activations x32 on the L0
    evacuation, and the PSUM descaled 2^-14 on the L1 evacuation
  - the final layer is pair-stacked feature-major; experts are combined
    by partition-sliced blend multiplies (DVE, reusing the bb broadcast
    tiles) and a [I64;I64] selector matmul that folds the two 64-row
    halves straight into a PSUM bank pre-loaded with the blended bias
  - weights are repacked host-side into [128, W] tensors with multi-KB
    contiguous per-partition lines and DMA'd in consumption order over
    several queues
  - the two 512-column macro-batches are software-pipelined: macro 1's
    gate matmuls cover macro 0's softmax/blend latency, the blend
    broadcasts for m0 are emitted ahead of m1's softmax tail so L0a can
    start the moment gate m1 ends (no HAM re-throttle), and macro 0's
    final layer is interleaved into macro 1's hidden-layer matmuls
"""
import sys

sys.path.insert(0, "/opt/trn_rl_repo")

import numpy as np

import concourse.bass as bass
import concourse.mybir as mybir
import concourse.tile as tile
from concourse.vector_clock import ScopedClock, VectorClock

F32 = mybir.dt.float32
F32R = mybir.dt.float32r
BF16 = mybir.dt.bfloat16
F8E4 = mybir.dt.float8e4
AF = mybir.ActivationFunctionType
ALU = mybir.AluOpType
DR = mybir.MatmulPerfMode.DoubleRow

B = 8192
OBS = 256
ACT2 = 64  # 2 * action_dim
HID = 512
P = 8
NCORES = 8
BL = B // NCORES          # batch per core = 1024
NB = 512                  # macro-batch (free-dim) size
NMACRO = BL // NB         # 2
KO = OBS // 128           # 2 obs k-chunks
KH = HID // 128           # 4 hidden k-chunks
NO = HID // 128           # 4 output chunks for HID layers
NWARM = 68                # HAM warm-up matmuls (N=64)

USE_FP8_L1 = True         # k-chunks 0,1 of L1 in fp8 DoubleRow
XSC = 32.0                # activation scale applied on the L0 evac
WSC = 512.0               # L1 weight scale (both halves)
DESC = 1.0 / (XSC * WSC)  # L1 evac descale


class _SplitDrainTileContext(tile.TileContext):
    """The walrus build in this container accepts very few sync-wait
    commands per instruction; the stock kernel-tail drain carries one wait
    per logical proc and fails codegen. Emit one SP nop per proc instead."""

    def _drain_and_barrier(self, tick_clock, wait_clock):
        gc = tick_clock.global_clock
        vec = list(gc)
        n = len(vec)
        for i, t in enumerate(vec):
            if t <= 0:
                continue
            sub = VectorClock([vec[j] if j == i else 0 for j in range(n)])
            nop = self.nc.sync.nop(nofuse=True)
            wait_clock.add_sem_waits(nop.ins, ScopedClock({None: sub}))
        self.nc.sync.drain()
        self.nc.all_engine_barrier()
        assert self.sems is not None
        popped = self.nc._tile_sem_poison_stack.pop()
        assert popped is self._sem_poison
        self.nc.clear_and_free_semaphores(list(self.sems.allocated().values()))
        self.nc.all_engine_barrier()


def _split_multi_waits(nc):
    """Hoist all but one sync-wait of each instruction onto NoOps on the
    same engine placed immediately before it (same queue => order kept)."""
    for f in nc.m.functions:
        for bb in f.blocks:
            new_insts = []
            for inst in bb.instructions:
                si = getattr(inst, "sync_info", None)
                ow = list(getattr(si, "on_wait", None) or [])
                if len(ow) > 1:
                    for w in ow[:-1]:
                        nop = mybir.InstNoOp(
                            name=f"I-waitsplit-{nc.next_id()}", ins=[], outs=[]
                        )
                        nop.engine = inst.engine
                        nop.sync_info = mybir.SyncInfo(on_wait=[w], on_update=[])
                        new_insts.append(nop)
                    si.on_wait = [ow[-1]]
                new_insts.append(inst)
            bb.instructions[:] = new_insts


def _build_program():
    nc = bass.Bass("TRN2", target_bir_lowering=False, debug=False)

    def din(name, shape, dtype=BF16):
        return nc.dram_tensor(name, shape, dtype, kind="ExternalInput").ap()

    # packed [128, W] weight tensors; col layout noted per tensor
    obs_w = din("obs_w", [128, KO * BL])          # chunk (m,k) at (m*KO+k)*NB
    gw0_w = din("gw0_w", [128, KO * HID])         # chunk k at cols k*HID
    gw1_w = din("gw1_w", [128, KH * HID])
    # gw2 zero-padded to M=128 so the gate-L2 matmuls keep the PE array
    # fully active (HAM reads low-occupancy matmuls as idle and
    # re-throttles the clock); same trick for the other K=8/M=8 matmuls
    gw2_w = din("gw2_w", [128, KH * 128])         # chunk k at cols k*128
    gb0_r = din("gb0_r", [128, NO], F32)          # col o = gb0[128o:128(o+1)]
    gb1_r = din("gb1_r", [128, NO], F32)
    gb2_c = din("gb2_c", [P, 1], F32)
    ew0_w = din("ew0_w", [128, KO * P * HID])     # chunk (k,p) at (k*P+p)*HID
    if USE_FP8_L1:
        # pair (k0,k1) per (p,o): 256 cols at (p*NO+o)*256, scaled x512
        ew1f8_w = din("ew1f8_w", [128, P * NO * 256], F8E4)
        # chunks k2,k3: chunk (k,p) at ((k-2)*P+p)*HID, scaled x512
        ew1b_w = din("ew1b_w", [128, 2 * P * HID])
    else:
        ew1_w = din("ew1_w", [128, KH * P * HID])
    ew2p_w = din("ew2p_w", [128, 4 * KH * 128])   # chunk (j,k) at (j*KH+k)*128
    eb0_m = din("eb0_m", [128, HID])              # rows 8-127 zero
    eb1_m = din("eb1_m", [128, HID])
    eb2_m = din("eb2_m", [128, 128])
    ones128 = din("ones128", [128, 128], BF16)
    onesM = din("onesM", [1, 128], F32)
    selB = din("selB", [128, P * 128])            # block p: rank-1 row-p sel
    sel64 = din("sel64", [128, 128])              # [I64; I64 | 0]

    out_t = nc.dram_tensor("out_t", [ACT2, BL], F32, kind="ExternalOutput").ap()

    with _SplitDrainTileContext(nc) as tc:
        with tc.tile_pool(name="wp", bufs=1) as wp, \
             tc.tile_pool(name="ap", bufs=2) as ap, \
             tc.tile_pool(name="xsp", bufs=6) as xsp, \
             tc.tile_pool(name="sp", bufs=2) as sp, \
             tc.tile_pool(name="pp", bufs=8, space="PSUM") as pp:

            # ---- load weights / consts, in consumption order ----
            def wtile(name, shape, src, dtype=BF16, nsplit=1, engs=None):
                engs = engs or [nc.sync]
                t = wp.tile(shape, dtype, name=name)
                w = shape[1] // nsplit
                for s in range(nsplit):
                    engs[s % len(engs)].dma_start(
                        t[:, s * w:(s + 1) * w], src[:, s * w:(s + 1) * w])
                return t

            # gate-critical path: the first ~1 MB is spread over four DGE
            # queues so the gate can start as soon as possible; the bulk
            # expert weights alternate the sync/scalar queues.  GpSimd
            # SWDGE carries the tiny consts.
            obs_sb = wp.tile([128, KO * BL], BF16, name="obs_sb")
            gw0_sb = wp.tile([128, KO * HID], BF16, name="gw0_sb")
            gw1_sb = wp.tile([128, KH * HID], BF16, name="gw1_sb")
            nc.sync.dma_start(gw0_sb[:, 0:HID], gw0_w[:, 0:HID])
            nc.scalar.dma_start(obs_sb[:, 0:NB], obs_w[:, 0:NB])
            nc.gpsimd.dma_start(gw0_sb[:, HID:2 * HID],
                                gw0_w[:, HID:2 * HID])
            nc.scalar.dma_start(obs_sb[:, NB:2 * NB], obs_w[:, NB:2 * NB])
            nc.sync.dma_start(gw1_sb[:, 0:HID], gw1_w[:, 0:HID])
            nc.gpsimd.dma_start(gw1_sb[:, HID:2 * HID],
                                gw1_w[:, HID:2 * HID])
            nc.sync.dma_start(gw1_sb[:, 2 * HID:3 * HID],
                              gw1_w[:, 2 * HID:3 * HID])
            nc.scalar.dma_start(gw1_sb[:, 3 * HID:4 * HID],
                                gw1_w[:, 3 * HID:4 * HID])
            nc.sync.dma_start(obs_sb[:, KO * NB:KO * BL],
                              obs_w[:, KO * NB:KO * BL])
            gw2_sb = wtile("gw2_sb", [128, KH * 128], gw2_w,
                           engs=[nc.gpsimd])
            gb0_sb = wtile("gb0_sb", [128, NO], gb0_r, F32, engs=[nc.gpsimd])
            gb1_sb = wtile("gb1_sb", [128, NO], gb1_r, F32, engs=[nc.gpsimd])
            gb2_sb = wtile("gb2_sb", [P, 1], gb2_c, F32, engs=[nc.gpsimd])
            ones128_sb = wtile("ones128_sb", [128, 128], ones128,
                               engs=[nc.gpsimd])
            onesM_sb = wtile("onesM_sb", [1, 128], onesM.bitcast(F32R), F32R,
                             engs=[nc.gpsimd])
            eb0_sb = wtile("eb0_sb", [128, HID], eb0_m, engs=[nc.gpsimd])
            selB_sb = wtile("selB_sb", [128, P * 128], selB,
                            engs=[nc.gpsimd])
            eb1_sb = wtile("eb1_sb", [128, HID], eb1_m, engs=[nc.gpsimd])
            # pre-warm the ACT LUTs (first use of each table costs ~1.3us)
            # while the bulk DMAs stream; keeps table loads off the
            # softmax/evac critical path
            twarm = wp.tile([128, 1], F32, name="twarm")
            nc.vector.memset(twarm[:], 1.0)
            for af in (AF.Exp, AF.Ln, AF.Relu, AF.Tanh):
                nc.scalar.activation(twarm[:], twarm[:], af)
            nc.scalar.copy(twarm[:], twarm[:])
            ew0_sb = wtile("ew0_sb", [128, KO * P * HID], ew0_w, nsplit=2,
                           engs=[nc.sync, nc.scalar])
            if USE_FP8_L1:
                ew1f8_sb = wtile("ew1f8_sb", [128, P * NO * 256], ew1f8_w,
                                 F8E4, nsplit=2, engs=[nc.sync, nc.scalar])
                ew1b_sb = wtile("ew1b_sb", [128, 2 * P * HID], ew1b_w,
                                nsplit=2, engs=[nc.sync, nc.scalar])
            else:
                ew1_sb = wtile("ew1_sb", [128, KH * P * HID], ew1_w, nsplit=4,
                               engs=[nc.sync, nc.scalar])
            ew2p_sb = wtile("ew2p_sb", [128, 4 * KH * 128], ew2p_w)
            sel64_sb = wtile("sel64_sb", [128, 128], sel64,
                             engs=[nc.gpsimd])
            eb2_sb = wtile("eb2_sb", [128, 128], eb2_m, engs=[nc.gpsimd])
            # zero-padded [128, NB] homes for the softmax tiles: rows 8-127
            # stay zero so full-K matmuls read exact zeros
            efm_t = {m: wp.tile([128, NB], BF16, name=f"efm128_{m}")
                     for m in range(2)}
            bfm_t = {m: wp.tile([128, NB], BF16, name=f"bfm128_{m}")
                     for m in range(2)}
            for m in range(2):
                nc.vector.memset(efm_t[m][:], 0.0)
                nc.vector.memset(bfm_t[m][:], 0.0)

            def ew0_c(p, k, o):
                c = (k * P + p) * HID + o * 128
                return ew0_sb[:, c:c + 128]

            def ew1b_c(p, k, o):
                # k in {2, 3}
                c = ((k - 2) * P + p) * HID + o * 128
                return ew1b_sb[:, c:c + 128]

            def ew1f8_c(p, o):
                c = (p * NO + o) * 256
                return ew1f8_sb[:, c:c + 256].rearrange(
                    "p (two f) -> p two f", two=2)

            def ew2p_c(j, k):
                c = (j * KH + k) * 128
                return ew2p_sb[:, c:c + 128]

            def obs_c(k, m):
                c = (m * KO + k) * NB
                return obs_sb[:, c:c + NB]

            neg15 = wp.tile([128, 1], F32, name="neg15")
            nc.vector.memset(neg15[:], -1.5)
            # HAM warm-up: keep the PE busy while the first DMAs land
            warm = wp.tile([128, 128], BF16, name="warm")
            nc.vector.memset(warm[:], 1.0)
            ps_w = pp.tile([128, 64], F32, name="ps_warm", tag="ps", bufs=4,
                           padded_shape=[128, NB])
            for _ in range(NWARM):
                nc.tensor.matmul(ps_w[:], warm[:], warm[:, 0:64],
                                 start=True, stop=True)

            # ---------------- gate network ----------------
            def gate_layer(m, lid, wsb, nk, xin, gb_sb):
                # k-outer emission: first matmuls need only the k=0 weight
                # chunk, so compute can start on partially-arrived DMAs
                pss = [pp.tile([128, NB], F32, name=f"ps_g{lid}_{o}",
                               tag="acc", bufs=4) for o in range(NO)]
                for k in range(nk):
                    for o in range(NO):
                        nc.tensor.matmul(
                            pss[o][:],
                            wsb[:, k * HID + o * 128:k * HID + (o + 1) * 128],
                            xin[k][:], start=(k == 0), stop=(k == nk - 1))
                xg = []
                for o in range(NO):
                    xt = ap.tile([128, NB], BF16, name=f"xg{lid}_{m}_{o}",
                                 bufs=1)
                    nc.vector.tensor_scalar(xt[:], pss[o][:],
                                            gb_sb[:, o:o + 1], 0.0,
                                            op0=ALU.add, op1=ALU.max)
                    xg.append(xt)
                return xg

            def build_gate(m, after_g0=None, after_g1=None):
                obs_m = [obs_c(k, m) for k in range(KO)]
                xg0 = gate_layer(m, 0, gw0_sb, KO, obs_m, gb0_sb)
                if after_g0 is not None:
                    after_g0()
                xg1 = gate_layer(m, 1, gw1_sb, KH, xg0, gb1_sb)
                if after_g1 is not None:
                    after_g1()
                ps_lg = pp.tile([128, NB], F32, name=f"ps_lg_{m}", tag="ps",
                                bufs=4)
                for k in range(KH):
                    nc.tensor.matmul(ps_lg[:],
                                     gw2_sb[:, k * 128:(k + 1) * 128],
                                     xg1[k][:], start=(k == 0), stop=False)
                e_fm = efm_t[m]
                nc.scalar.activation(e_fm[0:P, :], ps_lg[0:P, :], AF.Exp,
                                     bias=gb2_sb[:])
                return ps_lg, e_fm

            # softmax tail, split so each PE piece can be emitted separately
            def blend_sum(m, e_fm):
                ps_s = pp.tile([128, NB], F32, name=f"ps_s_{m}", tag="ps",
                               bufs=4)
                nc.tensor.matmul(ps_s[:], ones128_sb[:], e_fm[:],
                                 start=True, stop=True)
                return ps_s

            def blend_ln(m, ps_s):
                # log-softmax: lnS on ACT (~0.5us) beats the 3.3us DVE
                # single-lane iterative-divide reciprocal
                lns = sp.tile([1, NB], F32R, name=f"lns_{m}")
                with nc.allow_low_precision(reason="f32r storage for lnS"):
                    nc.scalar.activation(lns[:], ps_s[0:1, :], AF.Ln)
                return lns

            def blend_norm(m, ps_lg, lns):
                # accumulate -lnS onto the logits (onesM_sb is all -1)
                nc.tensor.matmul(ps_lg[:], onesM_sb[:], lns[:],
                                 start=False, stop=True)

            def blend_exp(m, ps_lg):
                blend_fm = bfm_t[m]
                nc.scalar.activation(blend_fm[0:P, :], ps_lg[0:P, :], AF.Exp,
                                     bias=gb2_sb[:])
                return blend_fm

            def bcast_one(m, blend_fm, p, out_list, copy_eng):
                # broadcast one blend row to 128 partitions via rank-1
                # selector matmul (full K=128) + ACT/DVE copy
                ps = pp.tile([128, NB], F32, name="ps_bc", tag="ps", bufs=4)
                nc.tensor.matmul(ps[:], selB_sb[:, p * 128:(p + 1) * 128],
                                 blend_fm[:], start=True, stop=True)
                bb = ap.tile([128, NB], BF16, name=f"bcb_{m}_{p}", bufs=1)
                if copy_eng == 0:
                    nc.scalar.copy(bb[:], ps[:])
                else:
                    nc.vector.tensor_copy(bb[:], ps[:])
                out_list.append(bb)

            # ------------- blended expert layers -------------
            def layer_acc(lname, eb_sb, blend_fm):
                """Allocate a layer's PSUM accumulators and emit the
                blended-bias matmuls (start=True)."""
                ps_l = []
                for o in range(NO):
                    ps = pp.tile([128, NB], F32, name=f"ps_{lname}_{o}",
                                 tag="acc", bufs=4)
                    nc.tensor.matmul(ps[:], eb_sb[:, o * 128:(o + 1) * 128],
                                     blend_fm[:], start=True, stop=False)
                    ps_l.append(ps)
                return ps_l

            def build_layer(m, lname, wsel, nk, xin, blendB, ps_l,
                            interleave=None, evac_scale=None):
                """One blended expert layer: out[o] = relu(sum_p W_p^T
                (x*b_p) + blend^T b).  interleave maps (k,p) chunk index ->
                fn emitting extra work between chunks."""
                ci = 0
                for k in range(nk):
                    for p in range(P):
                        if interleave and ci in interleave:
                            interleave[ci]()
                        ci += 1
                        xs = xsp.tile([128, NB], BF16, name="xs")
                        nc.vector.tensor_tensor(
                            xs[:], xin[k][:], blendB[p][:], op=ALU.mult)
                        for o in range(NO):
                            nc.tensor.matmul(
                                ps_l[o][:], wsel(p, k, o), xs[:],
                                start=False,
                                stop=(k == nk - 1 and p == P - 1))
                return _evac(lname, ps_l, evac_scale)

            def _evac(lname, ps_l, scale=None):
                xout = []
                for o in range(NO):
                    xt = ap.tile([128, NB], BF16, name=f"x_{lname[:2]}_{o}")
                    if scale is None:
                        if o % 2 == 0:
                            nc.scalar.activation(xt[:], ps_l[o][:], AF.Relu)
                        else:
                            nc.vector.tensor_scalar_max(xt[:], ps_l[o][:], 0.0)
                    else:
                        if o % 2 == 0:
                            nc.scalar.activation(xt[:], ps_l[o][:], AF.Relu,
                                                 scale=scale)
                        else:
                            nc.vector.tensor_scalar(xt[:], ps_l[o][:], scale,
                                                    0.0, op0=ALU.mult,
                                                    op1=ALU.max)
                    xout.append(xt)
                return xout

            def build_layer1(m, lname, xin, blendB, ps_l, interleave=None):
                """L1 with k-chunks 0,1 in fp8 DoubleRow (p-outer), chunks
                2,3 in bf16."""
                ci = 0
                for p in range(P):
                    if interleave and ci in interleave:
                        interleave[ci]()
                    ci += 1
                    xs8 = xsp.tile([128, 2 * NB], F8E4, name="xs8", bufs=3)
                    for k in range(2):
                        nc.vector.tensor_tensor(
                            xs8[:, k * NB:(k + 1) * NB], xin[k][:],
                            blendB[p][:], op=ALU.mult)
                    x3 = xs8[:].rearrange("p (two f) -> p two f", two=2)
                    for o in range(NO):
                        nc.tensor.matmul(ps_l[o][:], ew1f8_c(p, o), x3,
                                         start=False, stop=False,
                                         perf_mode=DR)
                    for k in range(2, KH):
                        if interleave and ci in interleave:
                            interleave[ci]()
                        ci += 1
                        xs = xsp.tile([128, NB], BF16, name="xs")
                        nc.vector.tensor_tensor(
                            xs[:], xin[k][:], blendB[p][:], op=ALU.mult)
                        for o in range(NO):
                            nc.tensor.matmul(
                                ps_l[o][:], ew1b_c(p, k, o), xs[:],
                                start=False,
                                stop=(k == KH - 1 and p == P - 1))
                return _evac(lname, ps_l, DESC)

            def build_l2_head(m, x2, c0=0, cw=NB):
                """Pair-stacked final-layer GEMMs + partition-sliced blend
                multiplies over columns [c0, c0+cw).  Returns tmul tiles;
                the fold matmuls are emitted later."""
                cs = slice(c0, c0 + cw)
                bb = bb_all[m]
                tmuls = []
                for j in range(4):
                    ps = pp.tile([128, cw], F32, name=f"ps2_{j}", tag="acc",
                                 bufs=4, padded_shape=[128, NB])
                    for k in range(KH):
                        nc.tensor.matmul(ps[:], ew2p_c(j, k), x2[k][:, cs],
                                         start=(k == 0), stop=(k == KH - 1))
                    tm = sp.tile([128, cw], BF16, name="tmul", bufs=3,
                                 padded_shape=[128, NB])
                    # experts 2j (rows 0:64) and 2j+1 (rows 64:128) scaled
                    # by their blend rows via slices of the bb tiles
                    nc.vector.tensor_tensor(tm[0:64, :], ps[0:64, :],
                                            bb[2 * j][0:64, cs],
                                            op=ALU.mult)
                    nc.vector.tensor_tensor(tm[64:128, :], ps[64:128, :],
                                            bb[2 * j + 1][64:128, cs],
                                            op=ALU.mult)
                    tmuls.append(tm)
                return tmuls

            def build_l2_fold(m, key, tmuls, blend_fm, js, c0=0, cw=NB):
                """Emit y-PSUM fold matmuls for pair indices js (0=bias)."""
                cs = slice(c0, c0 + cw)
                for j in js:
                    if j == 0:
                        y_ps[key] = pp.tile([128, cw], F32,
                                            name=f"ps_y_{key}", tag="ps",
                                            bufs=4, padded_shape=[128, NB])
                        nc.tensor.matmul(y_ps[key][:], eb2_sb[:],
                                         blend_fm[:, cs],
                                         start=True, stop=False)
                    nc.tensor.matmul(y_ps[key][:], sel64_sb[:], tmuls[j][:],
                                     start=False, stop=(j == 3))

            def build_l2_tail(m, key, c0=0, cw=NB):
                yp = y_ps[key]
                mu = sp.tile([32, cw], F32, name="mu_sb", bufs=2,
                             padded_shape=[32, NB])
                nc.vector.tensor_copy(mu[:], yp[0:32, :])
                tls = sp.tile([32, cw], F32, name="tls", bufs=2,
                              padded_shape=[32, NB])
                nc.scalar.activation(tls[:], yp[32:ACT2, :], AF.Tanh)
                stdt = sp.tile([32, cw], F32, name="stdt", bufs=2,
                               padded_shape=[32, NB])
                nc.scalar.activation(stdt[:], tls[:], AF.Exp,
                                     scale=3.5, bias=neg15[0:32, :])
                gc = slice(m * NB + c0, m * NB + c0 + cw)
                nc.sync.dma_start(out_t[0:32, gc], mu[:])
                nc.sync.dma_start(out_t[32:ACT2, gc], stdt[:])

            # ---------------- schedule ----------------
            y_ps = {}
            bb_all = {0: [], 1: []}
            hold = {}

            lg0, e0 = build_gate(0)                  # gates m0

            def _m0_sum():
                hold["s0"] = blend_sum(0, e0)

            def _m0_tail():
                hold["lns0"] = blend_ln(0, hold["s0"])
                blend_norm(0, lg0, hold["lns0"])
                hold["bf0"] = blend_exp(0, lg0)

            # gates m1 cover the m0 softmax chain
            lg1, e1 = build_gate(1, after_g0=_m0_sum, after_g1=_m0_tail)
            bf0 = hold["bf0"]
            bb0 = bb_all[0]
            # first two m0 blend broadcasts up front (their copies land on
            # ACT/DVE ahead of m1's softmax tail) so L0a's inputs are ready
            # the moment gate m1 ends; the rest interleave into L0a chunks
            bcast_one(0, bf0, 0, bb0, 0)
            bcast_one(0, bf0, 1, bb0, 1)
            acc_l0a = layer_acc("l0a", eb0_sb, bf0)
            ps_s1 = blend_sum(1, e1)
            lns1 = blend_ln(1, ps_s1)
            blend_norm(1, lg1, lns1)
            bf1 = blend_exp(1, lg1)
            hold["bf1"] = bf1
            bb_m1 = bb_all[1]
            il0 = {}
            for i in range(6):
                il0[i + 1] = (lambda p=i + 2, e=i % 2: bcast_one(0, bf0, p,
                                                                bb0, e))
            for i in range(P):
                il0[8 + i] = (lambda p=i, e=i % 2: bcast_one(1, bf1, p,
                                                             bb_m1, e))

            obs0 = [obs_c(k, 0) for k in range(KO)]
            obs1 = [obs_c(k, 1) for k in range(KO)]
            x1_0 = build_layer(0, "l0a", ew0_c, KO, obs0, bb0, acc_l0a,
                               interleave=il0,
                               evac_scale=XSC if USE_FP8_L1 else None)
            acc = layer_acc("l0b", eb0_sb, hold["bf1"])
            x1_1 = build_layer(1, "l0b", ew0_c, KO, obs1, bb_m1, acc,
                               evac_scale=XSC if USE_FP8_L1 else None)
            acc = layer_acc("l1a", eb1_sb, bf0)
            if USE_FP8_L1:
                x2_0 = build_layer1(0, "l1a", x1_0, bb0, acc)
            else:
                x2_0 = build_layer(0, "l1a", ew1_c, KH, x1_0, bb0, acc)

            tm0 = build_l2_head(0, x2_0)             # L2 m0 GEMMs
            build_l2_fold(0, "m0", tm0, bf0, [0, 1, 2])

            il1 = {2: (lambda: build_l2_fold(0, "m0", tm0, bf0, [3])),
                   4: (lambda: build_l2_tail(0, "m0"))}
            acc = layer_acc("l1b", eb1_sb, hold["bf1"])
            if USE_FP8_L1:
                x2_1 = build_layer1(1, "l1b", x1_1, bb_m1, acc,
                                    interleave=il1)
            else:
                x2_1 = build_layer(1, "l1b", ew1_c, KH, x1_1, bb_m1, acc,
                                   interleave=il1)

            # m1 final layer in two column chunks so the tmul/fold/tail
            # chain of chunk A hides under chunk B's GEMMs
            HW = NB // 2
            tmA = build_l2_head(1, x2_1, 0, HW)
            tmB = build_l2_head(1, x2_1, HW, HW)
            build_l2_fold(1, "m1a", tmA, hold["bf1"], [0, 1, 2, 3], 0, HW)
            build_l2_tail(1, "m1a", 0, HW)
            build_l2_fold(1, "m1b", tmB, hold["bf1"], [0, 1, 2, 3], HW, HW)
            build_l2_tail(1, "m1b", HW, HW)

    _split_multi_waits(nc)
    return nc


_NC_CACHE = None


def _get_program():
    global _NC_CACHE
    if _NC_CACHE is None:
        _NC_CACHE = _build_program()
    return _NC_CACHE


def _prep_core_inputs(inputs):
    import ml_dtypes
    f32 = np.float32
    bf16 = ml_dtypes.bfloat16
    f8 = ml_dtypes.float8_e4m3
    obs = np.ascontiguousarray(inputs["obs"], dtype=f32)
    gw0 = np.asarray(inputs["gw0"], f32)
    gb0 = np.asarray(inputs["gb0"], f32)
    gw1 = np.asarray(inputs["gw1"], f32)
    gb1 = np.asarray(inputs["gb1"], f32)
    gw2 = np.asarray(inputs["gw2"], f32)
    gb2 = np.asarray(inputs["gb2"], f32)
    ew0 = np.asarray(inputs["ew0"], f32)
    eb0 = np.asarray(inputs["eb0"], f32)
    ew1 = np.asarray(inputs["ew1"], f32)
    eb1 = np.asarray(inputs["eb1"], f32)
    ew2 = np.asarray(inputs["ew2"], f32)
    eb2 = np.asarray(inputs["eb2"], f32)

    def chunk_cols(a, nk):
        # [nk*128, W] -> [128, nk*W] with chunk k at cols [k*W:(k+1)*W]
        return np.concatenate([a[k * 128:(k + 1) * 128, :]
                               for k in range(nk)], axis=1)

    obs_tf = np.ascontiguousarray(obs.T)                      # [OBS, B] f32
    obs_bt = obs_tf.astype(bf16)
    gw0_w = np.ascontiguousarray(chunk_cols(gw0.T, KO).astype(bf16))
    gw1_w = np.ascontiguousarray(chunk_cols(gw1.T, KH).astype(bf16))
    # gw2 chunks zero-padded from M=8 to M=128
    gw2_t = gw2.T                                             # [HID, P]
    gw2_w = np.zeros((128, KH * 128), dtype=bf16)
    for k in range(KH):
        gw2_w[:, k * 128:k * 128 + P] = \
            gw2_t[k * 128:(k + 1) * 128, :].astype(bf16)
    gw2_w = np.ascontiguousarray(gw2_w)
    gb0_r = np.ascontiguousarray(gb0.reshape(NO, 128).T)      # [128, NO]
    gb1_r = np.ascontiguousarray(gb1.reshape(NO, 128).T)
    gb2_c = gb2.reshape(P, 1).astype(f32)
    # ew packed [128, nk*P*HID], chunk (k,p) at col (k*P+p)*HID
    ew0_t = ew0.transpose(0, 2, 1)                            # [P, OBS, HID]
    ew1_t = ew1.transpose(0, 2, 1)                            # [P, HID, HID]
    ew0_w = np.concatenate(
        [ew0_t[p, k * 128:(k + 1) * 128, :]
         for k in range(KO) for p in range(P)], axis=1).astype(bf16)
    if USE_FP8_L1:
        ew1s = ew1_t * WSC
        ew1f8_w = np.concatenate(
            [ew1s[p, k * 128:(k + 1) * 128, o * 128:(o + 1) * 128]
             for p in range(P) for o in range(NO) for k in range(2)],
            axis=1)
        ew1f8_w = np.clip(ew1f8_w, -240.0, 240.0).astype(f8)
        ew1b_w = np.concatenate(
            [ew1s[p, k * 128:(k + 1) * 128, :]
             for k in range(2, KH) for p in range(P)], axis=1).astype(bf16)
        eb1_m = (eb1 * (WSC * XSC)).astype(bf16)
    else:
        ew1_w = np.concatenate(
            [ew1_t[p, k * 128:(k + 1) * 128, :]
             for k in range(KH) for p in range(P)], axis=1).astype(bf16)
        eb1_m = eb1.astype(bf16)
    ew2_t = ew2.transpose(0, 2, 1)                            # [P, HID, 64]
    ew2p = [np.concatenate([ew2_t[2 * j], ew2_t[2 * j + 1]], axis=1)
            for j in range(4)]                                # [HID, 128] x4
    ew2p_w = np.concatenate(
        [ew2p[j][k * 128:(k + 1) * 128, :]
         for j in range(4) for k in range(KH)], axis=1).astype(bf16)
    ones128 = np.ones((128, 128), bf16)
    onesM = -np.ones((1, 128), f32)         # negated: accumulates -lnS
    selB = np.zeros((128, P * 128), bf16)
    for p in range(P):
        selB[p, p * 128:(p + 1) * 128] = 1.0
    sel64 = np.zeros((128, 128), bf16)
    for r in range(128):
        sel64[r, r % ACT2] = 1.0

    def padrows(a):
        out = np.zeros((128, a.shape[1]), dtype=bf16)
        out[:a.shape[0], :] = a.astype(bf16)
        return out

    eb2_p = np.zeros((128, 128), dtype=bf16)
    eb2_p[:P, :ACT2] = eb2.astype(bf16)

    shared = {
        "gw0_w": gw0_w, "gw1_w": gw1_w, "gw2_w": gw2_w,
        "gb0_r": gb0_r, "gb1_r": gb1_r, "gb2_c": gb2_c,
        "ew0_w": np.ascontiguousarray(ew0_w),
        "ew2p_w": np.ascontiguousarray(ew2p_w),
        "eb0_m": padrows(eb0), "eb1_m": padrows(eb1_m),
        "eb2_m": eb2_p,
        "ones128": ones128, "onesM": onesM, "selB": selB,
        "sel64": sel64,
    }
    if USE_FP8_L1:
        shared["ew1f8_w"] = np.ascontiguousarray(ew1f8_w)
        shared["ew1b_w"] = np.ascontiguousarray(ew1b_w)
    else:
        shared["ew1_w"] = np.ascontiguousarray(ew1_w)
    in_maps = []
    for c in range(NCORES):
        im = dict(shared)
        oc = obs_bt[:, c * BL:(c + 1) * BL]                   # [OBS, BL]
        im["obs_w"] = np.ascontiguousarray(
            np.concatenate([oc[k * 128:(k + 1) * 128, m * NB:(m + 1) * NB]
                            for m in range(NMACRO) for k in range(KO)],
                           axis=1))
        in_maps.append(im)
    return in_maps


def kernel(**inputs) -> np.ndarray:
    import time

    from concourse.bass_utils import run_bass_kernel_spmd

    nc = _get_program()
    in_maps = _prep_core_inputs(inputs)
    res = None
    last_err = None
    # a freshly-compiled NEFF occasionally hits a transient
    # NRT_EXEC_UNIT_UNRECOVERABLE on its first execution; a retry succeeds
    for attempt in range(3):
        try:
            res = run_bass_kernel_spmd(nc, in_maps, core_ids=list(range(NCORES)))
            break
        except Exception as e:  # noqa: BLE001
            last_err = e
            time.sleep(2.0)
    if res is None:
        raise last_err
    out = np.concatenate(
        [res.results[c]["out_t"].T for c in range(NCORES)], axis=0)
    return np.ascontiguousarray(out, dtype=np.float32)
